# revision 63
# baseline (speedup 1.0000x reference)
"""Trainium2 Bass kernel for nn_Block_54219667145535 (linear-attention block).

Sharding: 8 cores, 2 per batch (B=4). Each core computes k/v projection +
[D,D] kv state on its own 2048 tokens (pair-AllReduced in bf16), and
q/attention/FFN for its half of the sequence. All large matmuls run in
fp8 e4m3 with MatmulPerfMode.DoubleRow (2 k-chunks per instruction), with
per-tensor power-of-2 scales folded into the surrounding vector ops.
"""

import os
import sys
from contextlib import ExitStack

import numpy as np


def _ensure_paths():
    for p in ("/opt/trn_rl_repo", "/root/.axon_site/_ro/trn_rl_repo"):
        if os.path.isdir(p) and p not in sys.path:
            sys.path.insert(0, p)
    try:
        import concourse.bass  # noqa: F401
    except ImportError as e:  # pragma: no cover
        raise ImportError(f"concourse not importable: {e}")


_ensure_paths()

import ml_dtypes  # noqa: E402

import concourse.bass as bass  # noqa: E402
import concourse.bacc as bacc  # noqa: E402
import concourse.tile as tile  # noqa: E402
from concourse import mybir  # noqa: E402
from concourse.bass import ts  # noqa: E402
from concourse.masks import make_identity  # noqa: E402

F32 = mybir.dt.float32
F32R = mybir.dt.float32r
BF16 = mybir.dt.bfloat16
F8 = mybir.dt.float8e4
AF = mybir.ActivationFunctionType
ALU = mybir.AluOpType
DR = mybir.MatmulPerfMode.DoubleRow

E4NP = ml_dtypes.float8_e4m3
BFNP = ml_dtypes.bfloat16

D = 1024
DCH = 8  # d chunks of 128
H_PAD = 2816
HCH = 22  # h chunks of 128
LN_EPS = 1e-5
ATTN_EPS = 1e-6

WS = 128.0    # weight scale for wq/wk/wv/wg/wd
WS_U = 16.0   # weight scale for wu (act stored as 16*act, absmax ~75)
S_KV = 1.0 / 64.0  # kv|ksum half stored as val/64 (fp8 AllReduce sums halves)
KVW = 1040         # packed kv|ksum row: 1024 kv + 1 ksum + pad to 16B mult

FP8_DOWN = True  # down matmul in fp8 (else bf16)


def _bcast_row(nc, row_ap, parts=128):
    """AP that reads a [1, N] DRAM row replicated across `parts` partitions."""
    return bass.AP(
        tensor=row_ap.tensor,
        offset=row_ap.offset,
        ap=[[0, parts]] + [list(d) for d in row_ap.ap[1:]],
    )


def build_program(T_OWN=2048, n_cores=8):
    """Build the per-core Bass/Tile program. Returns (nc, input_names)."""
    assert T_OWN % 512 == 0
    NBLK = T_OWN // 512  # P1 blocks (own tokens only)
    NTG = T_OWN // 512  # P2 tgroups
    GROUPS = [[c, c + 1] for c in range(0, n_cores, 2)]
    WD_DT = F8 if FP8_DOWN else BF16
    # h2 = psd * DOWN_DESCALE + (x1 + bd)
    DOWN_DESCALE = 1.0 / (WS_U * WS) if FP8_DOWN else 1.0 / WS_U

    nc = bacc.Bacc(
        "TRN2",
        target_bir_lowering=False,
        debug=False,
        enable_asserts=False,
        num_devices=8,
        num_swdge_queues=4,
    )

    # ---- I/O ----
    x_ownT = nc.dram_tensor("x_ownT", [D, T_OWN], F8, kind="ExternalInput").ap()
    x_own = nc.dram_tensor("x_own", [T_OWN, D], F32, kind="ExternalInput").ap()
    wq = nc.dram_tensor("wq", [D, D], F8, kind="ExternalInput").ap()
    wk = nc.dram_tensor("wk", [D, D], F8, kind="ExternalInput").ap()
    wv = nc.dram_tensor("wv", [D, D], F8, kind="ExternalInput").ap()
    bq_pre = nc.dram_tensor("bq_pre", [128, DCH], F32, kind="ExternalInput").ap()
    bk_row = nc.dram_tensor("bk_row", [1, D], F32, kind="ExternalInput").ap()
    bv_row = nc.dram_tensor("bv_row", [1, D], F32, kind="ExternalInput").ap()
    wg = nc.dram_tensor("wg", [D, H_PAD], F8, kind="ExternalInput").ap()
    wu = nc.dram_tensor("wu", [D, H_PAD], F8, kind="ExternalInput").ap()
    bg_pre = nc.dram_tensor("bg_pre", [128, HCH], F32, kind="ExternalInput").ap()
    bu_pre = nc.dram_tensor("bu_pre", [128, HCH], F32, kind="ExternalInput").ap()
    wd = nc.dram_tensor("wd", [H_PAD, D], WD_DT, kind="ExternalInput").ap()
    bd_row = nc.dram_tensor("bd_row", [1, D], F32, kind="ExternalInput").ap()
    g1_row = nc.dram_tensor("g1_row", [1, D], BF16, kind="ExternalInput").ap()
    b1_row = nc.dram_tensor("b1_row", [1, D], BF16, kind="ExternalInput").ap()
    g2_row = nc.dram_tensor("g2_row", [1, D], BF16, kind="ExternalInput").ap()
    b2_row = nc.dram_tensor("b2_row", [1, D], BF16, kind="ExternalInput").ap()
    out = nc.dram_tensor("out", [T_OWN, D], F32, kind="ExternalOutput").ap()

    input_names = [
        "x_ownT", "x_own", "wq", "wk", "wv", "bq_pre", "bk_row",
        "bv_row", "wg", "wu", "bg_pre", "bu_pre", "wd", "bd_row",
        "g1_row", "b1_row", "g2_row", "b2_row",
    ]

    # d-chunked views of DRAM (partition-inner): [(c p) t -> p c t]
    x_ownT_v = x_ownT.rearrange("(c p) t -> p c t", p=128)
    wq_v = wq.rearrange("(c p) n -> p c n", p=128)
    wk_v = wk.rearrange("(c p) n -> p c n", p=128)
    wv_v = wv.rearrange("(c p) n -> p c n", p=128)
    wg_v = wg.rearrange("(c p) n -> p c n", p=128)
    wu_v = wu.rearrange("(c p) n -> p c n", p=128)
    wd_v = wd.rearrange("(c p) n -> p c n", p=128)

    with tile.TileContext(nc) as tc, ExitStack() as top:
        dram = top.enter_context(tc.tile_pool(name="dram", bufs=1, space="DRAM"))
        x1_dram = dram.tile([T_OWN, D], BF16, name="x1_dram")
        x1T_dram = dram.tile([D, T_OWN], F8, name="x1T_dram")
        x1T_dram_v = x1T_dram[:].rearrange("(c p) t -> p c t", p=128)

        # FFN weights are SBUF-resident for the whole kernel (fp8 makes them
        # small); their DMAs issue early in P1 and drain under P1/P2.
        wres = top.enter_context(tc.tile_pool(name="wres", bufs=1))
        wg_r = wres.tile([128, DCH, H_PAD], F8, name="wg_r")
        wu_r = wres.tile([128, DCH, H_PAD], F8, name="wu_r")

        consts = top.enter_context(tc.tile_pool(name="consts", bufs=1))
        ident = consts.tile([128, 128], BF16, name="ident")
        make_identity(nc, ident[:])
        ident2 = consts.tile([2, 2], F32, name="ident2")
        make_identity(nc, ident2[:])
        epsb = consts.tile([128, 1], F32, name="epsb")
        nc.vector.memset(epsb[:], LN_EPS)
        bq_s = consts.tile([128, DCH], F32, name="bq_s")
        nc.sync.dma_start(out=bq_s[:], in_=bq_pre)
        bqn_s = consts.tile([128, DCH], F32, name="bqn_s")
        nc.scalar.activation(bqn_s[:], bq_s[:], AF.Copy, scale=-1.0)
        bg_s = consts.tile([128, HCH], F32, name="bg_s")
        nc.sync.dma_start(out=bg_s[:], in_=bg_pre)
        bu_s = consts.tile([128, HCH], F32, name="bu_s")
        nc.sync.dma_start(out=bu_s[:], in_=bu_pre)

        # kv|ksum state (live P1..P2 only). ksum rides as column 1024 of
        # kv (ones-column appended to v), pad to 1040. Accumulated fully
        # in PSUM then quantized straight to fp8.
        p12 = top.enter_context(ExitStack())
        accs = p12.enter_context(tc.tile_pool(name="accs", bufs=1))
        kvks8 = accs.tile([128, DCH, KVW], F8, name="kvks8")
        kvks8_o = accs.tile([128, DCH, KVW], F8, name="kvks8_o")

        # ---------------- P1: k/v projection + kv/ksum over own tokens ----
        with ExitStack() as p1:
            c1_p = p1.enter_context(tc.tile_pool(name="c1", bufs=1))
            wkv_p = p1.enter_context(tc.tile_pool(name="wkv", bufs=1))
            xb_p = p1.enter_context(tc.tile_pool(name="xb", bufs=2))
            kpv_p = p1.enter_context(tc.tile_pool(name="kpv", bufs=NBLK))
            tmp_p = p1.enter_context(tc.tile_pool(name="p1tmp", bufs=3))
            ps_proj = p1.enter_context(
                tc.tile_pool(name="ps_proj", bufs=3, space="PSUM"))
            ps_kv = p1.enter_context(
                tc.tile_pool(name="ps_kv", bufs=3, space="PSUM"))

            # first x block before weights so PE can start ASAP
            xblk0 = xb_p.tile([128, DCH, 512], F8, name="xblk0", tag="xblk")
            for t4 in range(4):
                nc.sync.dma_start(out=xblk0[:, :, ts(t4, 128)],
                                  in_=x_ownT_v[:, :, ts(t4, 128)])
            wh = {}
            for which, half in ((0, 0), (1, 0), (0, 1), (1, 1)):
                w_v = wk_v if which == 0 else wv_v
                nm = f"w{'k' if which == 0 else 'v'}h{half}"
                t = wkv_p.tile([128, DCH, 512], F8, name=nm)
                for dc in range(DCH):
                    nc.scalar.dma_start(
                        out=t[:, dc, :],
                        in_=w_v[:, dc, ts(half, 512)])
                wh[(which, half)] = t
            bkb = c1_p.tile([128, D], F32, name="bkb")
            nc.sync.dma_start(out=bkb[:], in_=_bcast_row(nc, bk_row))
            bvb = c1_p.tile([128, D], F32, name="bvb")
            nc.sync.dma_start(out=bvb[:], in_=_bcast_row(nc, bv_row))

            # gate/up resident-weight loads: issued now so they drain in the
            # background during P1/P2 without delaying P1's first tiles
            for dc in range(DCH):
                nc.scalar.dma_start(out=wg_r[:, dc, :], in_=wg_v[:, dc, :])
                nc.scalar.dma_start(out=wu_r[:, dc, :], in_=wu_v[:, dc, :])

            kps, vls = [], []
            for blk in range(NBLK):
                if blk == 0:
                    xblk = xblk0
                else:
                    xblk = xb_p.tile([128, DCH, 512], F8, name=f"xblk{blk}",
                                     tag="xblk")
                    nc.sync.dma_start(out=xblk[:],
                                      in_=x_ownT_v[:, :, ts(blk, 512)])

                kp_blk = kpv_p.tile([128, 4, D], F8, name=f"kp{blk}", tag="kp")
                # v gets a ones column at 1024: the kv matmul then produces
                # ksum (= kp^T @ 1) as kv column 1024 for free
                v_blk = kpv_p.tile([128, 4, KVW], F8, name=f"v{blk}", tag="v")
                kps.append(kp_blk)
                vls.append(v_blk)
                nc.vector.memset(v_blk[:, :, D:D + 1], 1.0)
                nc.vector.memset(v_blk[:, :, D + 1:KVW], 0.0)

                for t4 in range(4):
                    for which, half in ((0, 0), (1, 0), (0, 1), (1, 1)):
                        w_s = wh[(which, half)]
                        gsl = ts(half, 512)
                        ps = ps_proj.tile([128, 512], F32,
                                          name=f"pp{blk}_{t4}_{which}_{half}",
                                          tag="ps_proj")
                        for dc2 in range(DCH // 2):
                            nc.tensor.matmul(
                                ps[:], xblk[:, 2 * dc2:2 * dc2 + 2,
                                            ts(t4, 128)],
                                w_s[:, 2 * dc2:2 * dc2 + 2, :],
                                start=(dc2 == 0), stop=(dc2 == DCH // 2 - 1),
                                perf_mode=DR)
                        if which == 0:
                            # k = ps/WS + bk;  kp = relu(k) + exp(min(k,0));
                            # min(k,0) = -relu(-k) keeps it all on Act engine
                            kb = tmp_p.tile([128, 512], F32,
                                            name=f"kb{blk}_{t4}_{half}", tag="kb")
                            nc.vector.scalar_tensor_tensor(
                                out=kb[:], in0=ps[:], scalar=1.0 / WS,
                                in1=bkb[:, gsl], op0=ALU.mult, op1=ALU.add)
                            rl = tmp_p.tile([128, 512], F32,
                                            name=f"rl{blk}_{t4}_{half}", tag="rl")
                            nc.scalar.activation(rl[:], kb[:], AF.Relu)
                            rn = tmp_p.tile([128, 512], F32,
                                            name=f"rn{blk}_{t4}_{half}", tag="rn")
                            nc.scalar.activation(rn[:], kb[:], AF.Relu,
                                                 scale=-1.0)
                            nc.scalar.activation(rn[:], rn[:], AF.Exp,
                                                 scale=-1.0)
                            nc.gpsimd.tensor_tensor(
                                out=kp_blk[:, t4, gsl], in0=rn[:],
                                in1=rl[:], op=ALU.add)
                        else:
                            nc.vector.scalar_tensor_tensor(
                                out=v_blk[:, t4, gsl], in0=ps[:],
                                scalar=1.0 / WS, in1=bvb[:, gsl],
                                op0=ALU.mult, op1=ALU.add)

            # kv phase: accumulate all 4 blocks (8 DoubleRow k-tiles) per
            # (dc, ec) in a single PSUM group, then quantize psum -> fp8 on
            # the Act engine. No SBUF accumulation traffic at all.
            for dc in range(DCH):
                dsl = ts(dc, 128)
                for ec, eo, ew in ((0, 0, 512), (1, 512, 512),
                                   (2, 1024, 16)):
                    esl = slice(eo, eo + ew)
                    pkv = ps_kv.tile([128, 512], F32,
                                     name=f"pkv{dc}_{ec}", tag="ps_kv")
                    for blk in range(NBLK):
                        for t4p in range(2):
                            nc.tensor.matmul(
                                pkv[:, 0:ew],
                                kps[blk][:, 2 * t4p:2 * t4p + 2, dsl],
                                vls[blk][:, 2 * t4p:2 * t4p + 2, esl],
                                start=(blk == 0 and t4p == 0),
                                stop=(blk == NBLK - 1 and t4p == 1),
                                perf_mode=DR)
                    nc.scalar.activation(kvks8[:, dc, esl], pkv[:, 0:ew],
                                         AF.Copy, scale=S_KV)

        # ---- pair AllReduce of (kv | ksum) in fp8 ------------------------
        # Each core's half is quantized to e4m3 (scaled) and the collective
        # sums the halves in fp8: the reduced buffer IS kv8/ksum8. The row
        # is padded to 1040 so the DoubleRow den matmul sees 16B strides.
        # pair AllReduce of the packed fp8 kv|ksum buffer (via DRAM; SBUF
        # collectives are broken in this stack)
        kv_ci = dram.tile([128, DCH, KVW], F8, name="kv_ci")
        kv_co = dram.tile([128, DCH, KVW], F8, name="kv_co")
        nc.sync.dma_start(out=kv_ci[:], in_=kvks8[:])
        nc.gpsimd.collective_compute(
            "AllReduce", ALU.add,
            ins=[kv_ci[:]], outs=[kv_co[:]], replica_groups=GROUPS)
        # result DMAs; ksum column first so den unblocks ASAP
        nc.gpsimd.dma_start(out=kvks8_o[:, :, D:D + 1],
                            in_=kv_co[:][:, :, D:D + 1])
        nc.gpsimd.dma_start(out=kvks8_o[:, :, 0:512],
                            in_=kv_co[:][:, :, 0:512])
        nc.gpsimd.dma_start(out=kvks8_o[:, :, 512:1024],
                            in_=kv_co[:][:, :, 512:1024])

        # ---------------- P2: q/num/den/attn/LN1/transpose per tgroup -----
        with ExitStack() as p2:
            c2_p = p2.enter_context(tc.tile_pool(name="c2", bufs=1))
            xg_p = p2.enter_context(tc.tile_pool(name="xg", bufs=2))
            qp_p = p2.enter_context(tc.tile_pool(name="qp", bufs=6))
            xtok_p = p2.enter_context(tc.tile_pool(name="xtok", bufs=4))
            h1_p = p2.enter_context(tc.tile_pool(name="h1", bufs=8))
            x1_p = p2.enter_context(tc.tile_pool(name="x1", bufs=4))
            x1f_p = p2.enter_context(tc.tile_pool(name="x1f", bufs=2))
            x1T_p = p2.enter_context(tc.tile_pool(name="x1T", bufs=2))
            tmp2_p = p2.enter_context(tc.tile_pool(name="p2tmp", bufs=3))
            st_p = p2.enter_context(tc.tile_pool(name="p2stat", bufs=4))
            den_p = p2.enter_context(tc.tile_pool(name="denp", bufs=2))
            ps_proj2 = p2.enter_context(
                tc.tile_pool(name="ps_proj2", bufs=3, space="PSUM"))
            ps_den = p2.enter_context(
                tc.tile_pool(name="ps_den", bufs=1, space="PSUM"))
            ps_num = p2.enter_context(
                tc.tile_pool(name="ps_num", bufs=2, space="PSUM"))
            ps_tr = p2.enter_context(
                tc.tile_pool(name="ps_tr", bufs=2, space="PSUM"))

            # q weights split into 4 column chunks (prefetch-friendly);
            # allocated last so the pool can be popped once projections done
            wq_sc = ExitStack()
            wq_p = wq_sc.enter_context(tc.tile_pool(name="wqp", bufs=4))
            wq_c = []
            for j in range(4):
                t = wq_p.tile([128, DCH, 256], F8, name=f"wq{j}", tag="wqc")
                nc.scalar.dma_start(out=t[:], in_=wq_v[:, :, ts(j, 256)])
                wq_c.append(t)
            g1b = c2_p.tile([128, D], BF16, name="g1b")
            nc.sync.dma_start(out=g1b[:], in_=_bcast_row(nc, g1_row))
            b1b = c2_p.tile([128, D], BF16, name="b1b")
            nc.sync.dma_start(out=b1b[:], in_=_bcast_row(nc, b1_row))

            def emit_qproj(tg):
                o = tg * 512
                xg = xg_p.tile([128, DCH, 512], F8, name=f"xg{tg}", tag="xg")
                nc.sync.dma_start(out=xg[:], in_=x_ownT_v[:, :, o:o + 512])
                qp_g = qp_p.tile([128, DCH, 512], F8, name=f"qpg{tg}", tag="qp")
                for qc in range(DCH):
                    ps = ps_proj2.tile([128, 512], F32, name=f"pq{tg}_{qc}",
                                       tag="ps_proj2")
                    for dc2 in range(DCH // 2):
                        nc.tensor.matmul(
                            ps[:],
                            wq_c[qc // 2][:, 2 * dc2:2 * dc2 + 2,
                                          ts(qc % 2, 128)],
                            xg[:, 2 * dc2:2 * dc2 + 2, :],
                            start=(dc2 == 0), stop=(dc2 == DCH // 2 - 1),
                            perf_mode=DR)
                    bql = bq_s[:, qc:qc + 1]
                    bqnl = bqn_s[:, qc:qc + 1]
                    # q = ps/WS + bq (bias applied inside Act, q is
                    # feature-major);  qp = relu(q) + exp(-relu(-q))
                    rl = tmp2_p.tile([128, 512], F32, name=f"qr{tg}_{qc}",
                                     tag="qr")
                    nc.scalar.activation(rl[:], ps[:], AF.Relu,
                                         bias=bql, scale=1.0 / WS)
                    rn = tmp2_p.tile([128, 512], F32, name=f"qn{tg}_{qc}",
                                     tag="qn")
                    nc.scalar.activation(rn[:], ps[:], AF.Relu,
                                         bias=bqnl, scale=-1.0 / WS)
                    nc.scalar.activation(rn[:], rn[:], AF.Exp, scale=-1.0)
                    # early tgroups on DVE (idle during the collective);
                    # late ones on gpsimd (its queue drains post-collective)
                    eng = nc.vector if tg < 4 else nc.gpsimd
                    eng.tensor_tensor(
                        out=qp_g[:, qc, :], in0=rn[:], in1=rl[:],
                        op=ALU.add)
                return qp_g

            # emit ALL q projections up front: they have no dependency on
            # the kv AllReduce, so their matmuls + activations fill the
            # collective's latency window.
            qp_queue = [emit_qproj(t) for t in range(NTG)]
            wq_sc.close()  # free q-weight SBUF once all projections queued

            for pr in range(0, NTG, 2):
                tgs = [t for t in (pr, pr + 1) if t < NTG]
                den_cs = {}
                for tg in tgs:
                    qp_g = qp_queue[tg]
                    # den*S for whole tgroup: [1, 512] = ksum8^T @ qp8
                    pdn = ps_den.tile([1, 512], F32, name=f"pdn{tg}",
                                      tag="ps_den")
                    for dc2 in range(DCH // 2):
                        nc.tensor.matmul(
                            pdn[:], kvks8_o[:, 2 * dc2:2 * dc2 + 2, D:D + 1],
                            qp_g[:, 2 * dc2:2 * dc2 + 2, :],
                            start=(dc2 == 0), stop=(dc2 == DCH // 2 - 1),
                            perf_mode=DR)
                    den_sb = den_p.tile([1, 512], F32, name=f"dnr{tg}",
                                        tag="dnr")
                    nc.vector.tensor_scalar_add(
                        out=den_sb[:], in0=pdn[:], scalar1=ATTN_EPS * S_KV)
                    nc.vector.reciprocal(out=den_sb[:], in_=den_sb[:])
                    # den_c = 1/(den*S+eps*S) = (1/S_KV)/(den_true+eps)
                    den_c = den_p.tile([128, 4, 1], F32, name=f"dnc{tg}",
                                       tag="dnc")
                    for t4 in range(4):
                        ptd = ps_tr.tile([128, 1], F32, name=f"ptd{tg}_{t4}",
                                         tag="ps_tr")
                        nc.tensor.transpose(ptd[:], den_sb[:, ts(t4, 128)],
                                            ident2[0:1, 0:1])
                        nc.scalar.copy(out=den_c[:, t4, :], in_=ptd[:])
                    den_cs[tg] = den_c

                # num in e-chunk phases: ec0 for both tgroups hides the
                # second AllReduce chunk; ec1 follows.
                h1s = {}
                for ec in range(2):
                    esl = ts(ec, 512)
                    for tg in tgs:
                        qp_g = qp_queue[tg]
                        o = tg * 512
                        for t4 in range(4):
                            tok = o + t4 * 128
                            if ec == 0:
                                h1s[(tg, t4)] = h1_p.tile(
                                    [128, D], BF16, name=f"h1_{tg}_{t4}",
                                    tag="h1")
                            xth = xtok_p.tile([128, 512], F32,
                                              name=f"xt{tg}_{t4}_{ec}",
                                              tag="xtok")
                            nc.sync.dma_start(
                                out=xth[:],
                                in_=x_own[tok:tok + 128, esl])
                            pn = ps_num.tile([128, 512], F32,
                                             name=f"pn{tg}_{t4}_{ec}",
                                             tag="ps_num")
                            for dc2 in range(DCH // 2):
                                nc.tensor.matmul(
                                    pn[:],
                                    qp_g[:, 2 * dc2:2 * dc2 + 2, ts(t4, 128)],
                                    kvks8_o[:, 2 * dc2:2 * dc2 + 2, esl],
                                    start=(dc2 == 0),
                                    stop=(dc2 == DCH // 2 - 1),
                                    perf_mode=DR)
                            nc.vector.scalar_tensor_tensor(
                                out=h1s[(tg, t4)][:, esl], in0=pn[:],
                                scalar=den_cs[tg][:, t4, 0:1],
                                in1=xth[:], op0=ALU.mult, op1=ALU.add)

                # LN1 + transpose
                for tg in tgs:
                    o = tg * 512
                    x1ns = []
                    for t4 in range(4):
                        tok = o + t4 * 128
                        h1 = h1s[(tg, t4)]
                        stats = st_p.tile([128, 2, 6], F32,
                                          name=f"s1_{tg}_{t4}", tag="st1")
                        nc.vector.bn_stats(out=stats[:, 0, :], in_=h1[:, 0:512])
                        nc.vector.bn_stats(out=stats[:, 1, :],
                                           in_=h1[:, 512:1024])
                        mv = st_p.tile([128, 2], F32, name=f"mv1_{tg}_{t4}",
                                       tag="mv1")
                        nc.vector.bn_aggr(out=mv[:], in_=stats[:])
                        rstd = st_p.tile([128, 1], F32, name=f"rs1_{tg}_{t4}",
                                         tag="rstd1")
                        nc.scalar.activation(rstd[:], mv[:, 1:2], AF.Sqrt,
                                             bias=epsb[:])
                        nc.vector.reciprocal(out=rstd[:], in_=rstd[:])
                        x1f = x1f_p.tile([128, D], BF16, name=f"x1f_{tg}_{t4}",
                                         tag="x1f")
                        nc.vector.tensor_scalar(
                            out=x1f[:], in0=h1[:], scalar1=mv[:, 0:1],
                            scalar2=rstd[:], op0=ALU.subtract, op1=ALU.mult)
                        nc.gpsimd.tensor_tensor(
                            out=x1f[:], in0=x1f[:], in1=g1b[:], op=ALU.mult)
                        x1n = x1_p.tile([128, D], BF16, name=f"x1_{tg}_{t4}",
                                        tag="x1")
                        nc.gpsimd.tensor_tensor(
                            out=x1n[:], in0=x1f[:], in1=b1b[:], op=ALU.add)
                        nc.sync.dma_start(out=x1_dram[tok:tok + 128, :],
                                          in_=x1n[:])
                        x1ns.append(x1n)

                    for t4 in range(4):
                        tok = o + t4 * 128
                        x1n = x1ns[t4]
                        x1T_t = x1T_p.tile([128, DCH, 128], F8,
                                           name=f"x1T{tg}_{t4}", tag="x1T")
                        for dc in range(DCH):
                            pt = ps_tr.tile([128, 128], BF16,
                                            name=f"pt{tg}_{t4}_{dc}",
                                            tag="ps_tr")
                            nc.tensor.transpose(pt[:], x1n[:, ts(dc, 128)],
                                                ident[:])
                            nc.scalar.copy(out=x1T_t[:, dc, :], in_=pt[:])
                        nc.sync.dma_start(
                            out=x1T_dram_v[:, :, tok:tok + 128], in_=x1T_t[:])

        p12.close()  # release kv/ksum accumulators before P3

        # ---------------- P3: FFN + LN2, gate/up and down pipelined -------
        # Per 512-token group: gate/up for all 22 h-chunks, then down +
        # LN2. Software-pipelined gu(0) gu(1) dn(0) gu(2) dn(1) gu(3)
        # dn(2) dn(3) so the PE never waits on the act8 chain.
        NTGH = T_OWN // 512
        with ExitStack() as p3:
            c3_p = p3.enter_context(tc.tile_pool(name="c3", bufs=1))
            wd_r = c3_p.tile([128, HCH, D], WD_DT, name="wd_r")
            for hc in range(HCH):
                nc.scalar.dma_start(out=wd_r[:, hc, :], in_=wd_v[:, hc, :])
            bdb = c3_p.tile([128, D], F32, name="bdb")
            nc.sync.dma_start(out=bdb[:], in_=_bcast_row(nc, bd_row))
            g2b = c3_p.tile([128, D], BF16, name="g2b")
            nc.sync.dma_start(out=g2b[:], in_=_bcast_row(nc, g2_row))
            b2b = c3_p.tile([128, D], BF16, name="b2b")
            nc.sync.dma_start(out=b2b[:], in_=_bcast_row(nc, b2_row))
            ffn_p = p3.enter_context(tc.tile_pool(name="ffn", bufs=3))
            x1T_q = p3.enter_context(tc.tile_pool(name="x1Tq", bufs=2))
            sg_p = p3.enter_context(tc.tile_pool(name="sg", bufs=2))
            x1r_p = p3.enter_context(tc.tile_pool(name="x1r", bufs=8))
            st3_p = p3.enter_context(tc.tile_pool(name="p3stat", bufs=4))
            out_p = p3.enter_context(tc.tile_pool(name="outp", bufs=2))
            ps_g = p3.enter_context(
                tc.tile_pool(name="ps_g", bufs=2, space="PSUM"))
            ps_u = p3.enter_context(
                tc.tile_pool(name="ps_u", bufs=2, space="PSUM"))
            ps_dn = p3.enter_context(
                tc.tile_pool(name="ps_dn", bufs=4, space="PSUM"))

            ffn_ts = {}

            def emit_gu(tgh):
                o = tgh * 512
                x1T_t = x1T_q.tile([128, DCH, 512], F8, name=f"x1Tq{tgh}",
                                   tag="x1Tq")
                nc.sync.dma_start(out=x1T_t[:],
                                  in_=x1T_dram_v[:, :, o:o + 512])
                ffn_t = ffn_p.tile([128, HCH, 512], F8 if FP8_DOWN else BF16,
                                   name=f"ffn{tgh}", tag="ffn")
                ffn_ts[tgh] = ffn_t
                for hc in range(HCH):
                    psg = ps_g.tile([128, 512], F32, name=f"pg{tgh}_{hc}",
                                    tag="ps_g")
                    for dc2 in range(DCH // 2):
                        nc.tensor.matmul(
                            psg[:],
                            wg_r[:, 2 * dc2:2 * dc2 + 2, ts(hc, 128)],
                            x1T_t[:, 2 * dc2:2 * dc2 + 2, :],
                            start=(dc2 == 0), stop=(dc2 == DCH // 2 - 1),
                            perf_mode=DR)
                    psu = ps_u.tile([128, 512], F32, name=f"pu{tgh}_{hc}",
                                    tag="ps_u")
                    for dc2 in range(DCH // 2):
                        nc.tensor.matmul(
                            psu[:],
                            wu_r[:, 2 * dc2:2 * dc2 + 2, ts(hc, 128)],
                            x1T_t[:, 2 * dc2:2 * dc2 + 2, :],
                            start=(dc2 == 0), stop=(dc2 == DCH // 2 - 1),
                            perf_mode=DR)
                    # silu(gate) on Act; act8 = (psu+16*bu)*silu
                    # (stored act is 16x true act)
                    sig = sg_p.tile([128, 512], F32, name=f"sig{tgh}_{hc}",
                                    tag="sig")
                    nc.scalar.activation(
                        sig[:], psg[:], AF.Silu,
                        bias=bg_s[:, hc:hc + 1], scale=1.0 / WS)
                    nc.vector.scalar_tensor_tensor(
                        out=ffn_t[:, hc, :], in0=psu[:],
                        scalar=bu_s[:, hc:hc + 1], in1=sig[:],
                        op0=ALU.add, op1=ALU.mult)

            def emit_down(tgh):
                ffn_t = ffn_ts[tgh]
                o = tgh * 512
                x1r = []
                for t8 in range(4):
                    tok = o + t8 * 128
                    xr = x1r_p.tile([128, D], BF16, name=f"x1r{tgh}_{t8}",
                                    tag="x1r")
                    nc.gpsimd.dma_start(out=xr[:],
                                        in_=x1_dram[tok:tok + 128, :])
                    nc.gpsimd.tensor_tensor(
                        out=xr[:], in0=xr[:], in1=bdb[:], op=ALU.add)
                    x1r.append(xr)
                for dg in range(2):
                    dsl = ts(dg, 512)
                    psd = [ps_dn.tile([128, 512], F32,
                                      name=f"pd{tgh}_{dg}_{t8}", tag="ps_dn")
                           for t8 in range(4)]
                    for hc2 in range(HCH // 2):
                        for t8 in range(4):
                            if FP8_DOWN:
                                nc.tensor.matmul(
                                    psd[t8][:],
                                    ffn_t[:, 2 * hc2:2 * hc2 + 2,
                                          ts(t8, 128)],
                                    wd_r[:, 2 * hc2:2 * hc2 + 2, dsl],
                                    start=(hc2 == 0),
                                    stop=(hc2 == HCH // 2 - 1),
                                    perf_mode=DR)
                            else:
                                for hi in range(2):
                                    hc = 2 * hc2 + hi
                                    nc.tensor.matmul(
                                        psd[t8][:],
                                        ffn_t[:, hc, ts(t8, 128)],
                                        wd_r[:, hc, dsl],
                                        start=(hc == 0),
                                        stop=(hc == HCH - 1))
                    for t8 in range(4):
                        # h2 = psd*descale + (x1 + bd)  (in-place)
                        nc.vector.scalar_tensor_tensor(
                            out=x1r[t8][:, dsl], in0=psd[t8][:],
                            scalar=DOWN_DESCALE, in1=x1r[t8][:, dsl],
                            op0=ALU.mult, op1=ALU.add)

                # LN2 + store for this group
                for t8 in range(4):
                    tok = o + t8 * 128
                    h2 = x1r[t8]
                    stats = st3_p.tile([128, 2, 6], F32,
                                       name=f"s2_{tgh}_{t8}", tag="st2")
                    nc.vector.bn_stats(out=stats[:, 0, :], in_=h2[:, 0:512])
                    nc.vector.bn_stats(out=stats[:, 1, :],
                                       in_=h2[:, 512:1024])
                    mv = st3_p.tile([128, 2], F32, name=f"mv2_{tgh}_{t8}",
                                    tag="mv2")
                    nc.vector.bn_aggr(out=mv[:], in_=stats[:])
                    rstd = st3_p.tile([128, 1], F32, name=f"rs2_{tgh}_{t8}",
                                      tag="rstd2")
                    nc.scalar.activation(rstd[:], mv[:, 1:2], AF.Sqrt,
                                         bias=epsb[:])
                    nc.vector.reciprocal(out=rstd[:], in_=rstd[:])
                    o_t = out_p.tile([128, D], BF16, name=f"o{tgh}_{t8}",
                                     tag="ot")
                    nc.vector.tensor_scalar(
                        out=o_t[:], in0=h2[:], scalar1=mv[:, 0:1],
                        scalar2=rstd[:], op0=ALU.subtract, op1=ALU.mult)
                    nc.vector.tensor_tensor(
                        out=o_t[:], in0=o_t[:], in1=g2b[:], op=ALU.mult)
                    of = out_p.tile([128, D], F32, name=f"of{tgh}_{t8}",
                                    tag="of")
                    nc.gpsimd.tensor_tensor(
                        out=of[:], in0=o_t[:], in1=b2b[:], op=ALU.add)
                    nc.sync.dma_start(out=out[tok:tok + 128, :], in_=of[:])

            emit_gu(0)
            for tgh in range(1, NTGH):
                emit_gu(tgh)
                emit_down(tgh - 1)
            emit_down(NTGH - 1)

    nc.compile()
    return nc, input_names


# ---------------------------------------------------------------------------
# Host-side wrapper
# ---------------------------------------------------------------------------

B, S, D_MODEL, D_FF = 4, 4096, 1024, 4096
FFN_H = int(2 * D_FF / 3)  # 2730

_cache = {}
LAST_RESULTS = None


def _get_program(T_OWN=2048, T_FULL=4096):
    key = (T_OWN, T_FULL)
    if key not in _cache:
        _cache[key] = build_program(T_OWN, T_FULL)
    return _cache[key]


def _prep_shared(Wqkv, bqkv, Wg, bg, Wu, bu, Wd, bd, g1, b1, g2, b2):
    f = np.float32
    Wqkv = np.asarray(Wqkv, f)
    sh = {}
    sh["wq"] = np.asarray(Wqkv[:, 0:1024] * WS, E4NP)
    sh["wk"] = np.asarray(Wqkv[:, 1024:2048] * WS, E4NP)
    sh["wv"] = np.asarray(Wqkv[:, 2048:3072] * WS, E4NP)
    bqkv = np.asarray(bqkv, f)
    sh["bq_pre"] = np.ascontiguousarray(bqkv[0:1024].reshape(8, 128).T)
    sh["bk_row"] = np.ascontiguousarray(bqkv[1024:2048].reshape(1, 1024))
    sh["bv_row"] = np.ascontiguousarray(bqkv[2048:3072].reshape(1, 1024))
    wg_p = np.zeros((1024, H_PAD), f)
    wg_p[:, :FFN_H] = np.asarray(Wg, f)
    sh["wg"] = np.asarray(wg_p * WS, E4NP)
    wu_p = np.zeros((1024, H_PAD), f)
    wu_p[:, :FFN_H] = np.asarray(Wu, f)
    sh["wu"] = np.asarray(wu_p * WS_U, E4NP)
    bg_p = np.zeros((H_PAD,), f)
    bg_p[:FFN_H] = np.asarray(bg, f)
    sh["bg_pre"] = np.ascontiguousarray(bg_p.reshape(HCH, 128).T)
    bu_p = np.zeros((H_PAD,), f)
    bu_p[:FFN_H] = np.asarray(bu, f) * WS_U  # stored bias is 16*bu
    sh["bu_pre"] = np.ascontiguousarray(bu_p.reshape(HCH, 128).T)
    wd_p = np.zeros((H_PAD, 1024), f)
    wd_p[:FFN_H, :] = np.asarray(Wd, f)
    if FP8_DOWN:
        sh["wd"] = np.asarray(wd_p * WS, E4NP)
    else:
        sh["wd"] = np.asarray(wd_p, BFNP)
    sh["bd_row"] = np.asarray(bd, f).reshape(1, 1024)
    sh["g1_row"] = np.asarray(g1, BFNP).reshape(1, 1024)
    sh["b1_row"] = np.asarray(b1, BFNP).reshape(1, 1024)
    sh["g2_row"] = np.asarray(g2, BFNP).reshape(1, 1024)
    sh["b2_row"] = np.asarray(b2, BFNP).reshape(1, 1024)
    return sh


def make_in_maps(x, Wqkv, bqkv, Wg, bg, Wu, bu, Wd, bd, g1, b1, g2, b2):
    x = np.asarray(x, np.float32)
    sh = _prep_shared(Wqkv, bqkv, Wg, bg, Wu, bu, Wd, bd, g1, b1, g2, b2)
    x8 = np.asarray(x, E4NP)
    in_maps = []
    for c in range(8):
        b, h = c // 2, c % 2
        m = dict(sh)
        m["x_ownT"] = np.ascontiguousarray(x8[b, h * 2048:(h + 1) * 2048].T)
        m["x_own"] = np.ascontiguousarray(x[b, h * 2048:(h + 1) * 2048])
        in_maps.append(m)
    return in_maps


def kernel(x, Wqkv, bqkv, Wg, bg, Wu, bu, Wd, bd, g1, b1, g2, b2):
    global LAST_RESULTS
    from concourse import bass_utils

    nc, _names = _get_program()
    in_maps = make_in_maps(x, Wqkv, bqkv, Wg, bg, Wu, bu, Wd, bd,
                           g1, b1, g2, b2)
    res = bass_utils.run_bass_kernel_spmd(nc, in_maps, core_ids=list(range(8)))
    LAST_RESULTS = res
    out = np.empty((B, S, D_MODEL), np.float32)
    for c in range(8):
        b, h = c // 2, c % 2
        out[b, h * 2048:(h + 1) * 2048] = res.results[c]["out"]
    return out


# revision 64
# speedup vs baseline: 1.0544x; 1.0544x over previous
"""Trainium2 Bass kernel for nn_Block_54219667145535 (linear-attention block).

Sharding: 8 cores, 2 per batch (B=4). Each core computes k/v projection +
[D,D] kv state on its own 2048 tokens (pair-AllReduced in bf16), and
q/attention/FFN for its half of the sequence. All large matmuls run in
fp8 e4m3 with MatmulPerfMode.DoubleRow (2 k-chunks per instruction), with
per-tensor power-of-2 scales folded into the surrounding vector ops.
"""

import os
import sys
from contextlib import ExitStack

import numpy as np


def _ensure_paths():
    for p in ("/opt/trn_rl_repo", "/root/.axon_site/_ro/trn_rl_repo"):
        if os.path.isdir(p) and p not in sys.path:
            sys.path.insert(0, p)
    try:
        import concourse.bass  # noqa: F401
    except ImportError as e:  # pragma: no cover
        raise ImportError(f"concourse not importable: {e}")


_ensure_paths()

import ml_dtypes  # noqa: E402

import concourse.bass as bass  # noqa: E402
import concourse.bacc as bacc  # noqa: E402
import concourse.tile as tile  # noqa: E402
from concourse import mybir  # noqa: E402
from concourse.bass import ts  # noqa: E402
from concourse.masks import make_identity  # noqa: E402

F32 = mybir.dt.float32
F32R = mybir.dt.float32r
BF16 = mybir.dt.bfloat16
F8 = mybir.dt.float8e4
AF = mybir.ActivationFunctionType
ALU = mybir.AluOpType
DR = mybir.MatmulPerfMode.DoubleRow

E4NP = ml_dtypes.float8_e4m3
BFNP = ml_dtypes.bfloat16

D = 1024
DCH = 8  # d chunks of 128
H_PAD = 2816
HCH = 22  # h chunks of 128
LN_EPS = 1e-5
ATTN_EPS = 1e-6

WS = 128.0    # weight scale for wq/wk/wv/wg/wd
WS_U = 16.0   # weight scale for wu (act stored as 16*act, absmax ~75)
S_KV = 1.0 / 64.0  # kv|ksum half stored as val/64 (fp8 AllReduce sums halves)
KVW = 1040         # packed kv|ksum row: 1024 kv + 1 ksum + pad to 16B mult

FP8_DOWN = True  # down matmul in fp8 (else bf16)


def _bcast_row(nc, row_ap, parts=128):
    """AP that reads a [1, N] DRAM row replicated across `parts` partitions."""
    return bass.AP(
        tensor=row_ap.tensor,
        offset=row_ap.offset,
        ap=[[0, parts]] + [list(d) for d in row_ap.ap[1:]],
    )


def build_program(T_OWN=2048, n_cores=8):
    """Build the per-core Bass/Tile program. Returns (nc, input_names)."""
    assert T_OWN % 512 == 0
    NBLK = T_OWN // 512  # P1 blocks (own tokens only)
    NTG = T_OWN // 512  # P2 tgroups
    GROUPS = [[c, c + 1] for c in range(0, n_cores, 2)]
    WD_DT = F8 if FP8_DOWN else BF16
    # h2 = psd * DOWN_DESCALE + (x1 + bd)
    DOWN_DESCALE = 1.0 / (WS_U * WS) if FP8_DOWN else 1.0 / WS_U

    nc = bacc.Bacc(
        "TRN2",
        target_bir_lowering=False,
        debug=False,
        enable_asserts=False,
        num_devices=8,
        num_swdge_queues=4,
    )

    # ---- I/O ----
    x_ownT = nc.dram_tensor("x_ownT", [D, T_OWN], F8, kind="ExternalInput").ap()
    x_own = nc.dram_tensor("x_own", [T_OWN, D], F32, kind="ExternalInput").ap()
    wq = nc.dram_tensor("wq", [D, D], F8, kind="ExternalInput").ap()
    wk = nc.dram_tensor("wk", [D, D], F8, kind="ExternalInput").ap()
    wv = nc.dram_tensor("wv", [D, D], F8, kind="ExternalInput").ap()
    bq_pre = nc.dram_tensor("bq_pre", [128, DCH], F32, kind="ExternalInput").ap()
    bk_row = nc.dram_tensor("bk_row", [1, D], F32, kind="ExternalInput").ap()
    bv_row = nc.dram_tensor("bv_row", [1, D], F32, kind="ExternalInput").ap()
    wg = nc.dram_tensor("wg", [D, H_PAD], F8, kind="ExternalInput").ap()
    wu = nc.dram_tensor("wu", [D, H_PAD], F8, kind="ExternalInput").ap()
    bg_pre = nc.dram_tensor("bg_pre", [128, HCH], F32, kind="ExternalInput").ap()
    bu_pre = nc.dram_tensor("bu_pre", [128, HCH], F32, kind="ExternalInput").ap()
    wd = nc.dram_tensor("wd", [H_PAD, D], WD_DT, kind="ExternalInput").ap()
    bd_row = nc.dram_tensor("bd_row", [1, D], F32, kind="ExternalInput").ap()
    g1_row = nc.dram_tensor("g1_row", [1, D], BF16, kind="ExternalInput").ap()
    b1_row = nc.dram_tensor("b1_row", [1, D], BF16, kind="ExternalInput").ap()
    g2_row = nc.dram_tensor("g2_row", [1, D], BF16, kind="ExternalInput").ap()
    b2_row = nc.dram_tensor("b2_row", [1, D], BF16, kind="ExternalInput").ap()
    out = nc.dram_tensor("out", [T_OWN, D], F32, kind="ExternalOutput").ap()

    input_names = [
        "x_ownT", "x_own", "wq", "wk", "wv", "bq_pre", "bk_row",
        "bv_row", "wg", "wu", "bg_pre", "bu_pre", "wd", "bd_row",
        "g1_row", "b1_row", "g2_row", "b2_row",
    ]

    # d-chunked views of DRAM (partition-inner): [(c p) t -> p c t]
    x_ownT_v = x_ownT.rearrange("(c p) t -> p c t", p=128)
    wq_v = wq.rearrange("(c p) n -> p c n", p=128)
    wk_v = wk.rearrange("(c p) n -> p c n", p=128)
    wv_v = wv.rearrange("(c p) n -> p c n", p=128)
    wg_v = wg.rearrange("(c p) n -> p c n", p=128)
    wu_v = wu.rearrange("(c p) n -> p c n", p=128)
    wd_v = wd.rearrange("(c p) n -> p c n", p=128)

    with tile.TileContext(nc) as tc, ExitStack() as top:
        dram = top.enter_context(tc.tile_pool(name="dram", bufs=1, space="DRAM"))
        x1_dram = dram.tile([T_OWN, D], BF16, name="x1_dram")
        x1T_dram = dram.tile([D, T_OWN], F8, name="x1T_dram")
        x1T_dram_v = x1T_dram[:].rearrange("(c p) t -> p c t", p=128)

        # FFN weights are SBUF-resident for the whole kernel (fp8 makes them
        # small); their DMAs issue early in P1 and drain under P1/P2.
        wres = top.enter_context(tc.tile_pool(name="wres", bufs=1))
        wg_r = wres.tile([128, DCH, H_PAD], F8, name="wg_r")
        wu_r = wres.tile([128, DCH, H_PAD], F8, name="wu_r")

        consts = top.enter_context(tc.tile_pool(name="consts", bufs=1))
        ident = consts.tile([128, 128], BF16, name="ident")
        make_identity(nc, ident[:])
        ident2 = consts.tile([2, 2], F32, name="ident2")
        make_identity(nc, ident2[:])
        epsb = consts.tile([128, 1], F32, name="epsb")
        nc.vector.memset(epsb[:], LN_EPS)
        bq_s = consts.tile([128, DCH], F32, name="bq_s")
        nc.sync.dma_start(out=bq_s[:], in_=bq_pre)
        bqn_s = consts.tile([128, DCH], F32, name="bqn_s")
        nc.scalar.activation(bqn_s[:], bq_s[:], AF.Copy, scale=-1.0)
        bg_s = consts.tile([128, HCH], F32, name="bg_s")
        nc.sync.dma_start(out=bg_s[:], in_=bg_pre)
        bu_s = consts.tile([128, HCH], F32, name="bu_s")
        nc.sync.dma_start(out=bu_s[:], in_=bu_pre)

        # kv|ksum state (live P1..P2 only). ksum rides as column 1024 of
        # kv (ones-column appended to v), pad to 1040. Accumulated fully
        # in PSUM then quantized straight to fp8.
        p12 = top.enter_context(ExitStack())
        accs = p12.enter_context(tc.tile_pool(name="accs", bufs=1))
        kvks8 = accs.tile([128, DCH, KVW], F8, name="kvks8")
        kvks8_o = accs.tile([128, DCH, KVW], F8, name="kvks8_o")

        # ---------------- P1: k/v projection + kv/ksum over own tokens ----
        with ExitStack() as p1:
            c1_p = p1.enter_context(tc.tile_pool(name="c1", bufs=1))
            wkv_p = p1.enter_context(tc.tile_pool(name="wkv", bufs=1))
            xb_p = p1.enter_context(tc.tile_pool(name="xb", bufs=2))
            kpv_p = p1.enter_context(tc.tile_pool(name="kpv", bufs=NBLK))
            tmp_p = p1.enter_context(tc.tile_pool(name="p1tmp", bufs=3))
            ps_proj = p1.enter_context(
                tc.tile_pool(name="ps_proj", bufs=3, space="PSUM"))
            ps_kv = p1.enter_context(
                tc.tile_pool(name="ps_kv", bufs=3, space="PSUM"))

            # first x block before weights so PE can start ASAP
            xblk0 = xb_p.tile([128, DCH, 512], F8, name="xblk0", tag="xblk")
            for t4 in range(4):
                nc.sync.dma_start(out=xblk0[:, :, ts(t4, 128)],
                                  in_=x_ownT_v[:, :, ts(t4, 128)])
            wh = {}
            for which, half in ((0, 0), (1, 0), (0, 1), (1, 1)):
                w_v = wk_v if which == 0 else wv_v
                nm = f"w{'k' if which == 0 else 'v'}h{half}"
                t = wkv_p.tile([128, DCH, 512], F8, name=nm)
                for dc in range(DCH):
                    nc.scalar.dma_start(
                        out=t[:, dc, :],
                        in_=w_v[:, dc, ts(half, 512)])
                wh[(which, half)] = t
            bkb = c1_p.tile([128, D], F32, name="bkb")
            nc.sync.dma_start(out=bkb[:], in_=_bcast_row(nc, bk_row))
            bvb = c1_p.tile([128, D], F32, name="bvb")
            nc.sync.dma_start(out=bvb[:], in_=_bcast_row(nc, bv_row))

            # gate/up resident-weight loads: issued now so they drain in the
            # background during P1/P2 without delaying P1's first tiles
            for dc in range(DCH):
                nc.scalar.dma_start(out=wg_r[:, dc, :], in_=wg_v[:, dc, :])
                nc.scalar.dma_start(out=wu_r[:, dc, :], in_=wu_v[:, dc, :])

            kps, vls = [], []
            for blk in range(NBLK):
                if blk == 0:
                    xblk = xblk0
                else:
                    xblk = xb_p.tile([128, DCH, 512], F8, name=f"xblk{blk}",
                                     tag="xblk")
                    nc.sync.dma_start(out=xblk[:],
                                      in_=x_ownT_v[:, :, ts(blk, 512)])

                kp_blk = kpv_p.tile([128, 4, D], F8, name=f"kp{blk}", tag="kp")
                # v gets a ones column at 1024: the kv matmul then produces
                # ksum (= kp^T @ 1) as kv column 1024 for free
                v_blk = kpv_p.tile([128, 4, KVW], F8, name=f"v{blk}", tag="v")
                kps.append(kp_blk)
                vls.append(v_blk)
                nc.vector.memset(v_blk[:, :, D:D + 1], 1.0)
                nc.vector.memset(v_blk[:, :, D + 1:KVW], 0.0)

                for t4 in range(4):
                    for which, half in ((0, 0), (1, 0), (0, 1), (1, 1)):
                        w_s = wh[(which, half)]
                        gsl = ts(half, 512)
                        ps = ps_proj.tile([128, 512], F32,
                                          name=f"pp{blk}_{t4}_{which}_{half}",
                                          tag="ps_proj")
                        for dc2 in range(DCH // 2):
                            nc.tensor.matmul(
                                ps[:], xblk[:, 2 * dc2:2 * dc2 + 2,
                                            ts(t4, 128)],
                                w_s[:, 2 * dc2:2 * dc2 + 2, :],
                                start=(dc2 == 0), stop=(dc2 == DCH // 2 - 1),
                                perf_mode=DR)
                        if which == 0:
                            # k = ps/WS + bk;  kp = relu(k) + exp(min(k,0));
                            # min(k,0) = -relu(-k) keeps it all on Act engine
                            kb = tmp_p.tile([128, 512], F32,
                                            name=f"kb{blk}_{t4}_{half}", tag="kb")
                            nc.vector.scalar_tensor_tensor(
                                out=kb[:], in0=ps[:], scalar=1.0 / WS,
                                in1=bkb[:, gsl], op0=ALU.mult, op1=ALU.add)
                            rl = tmp_p.tile([128, 512], F32,
                                            name=f"rl{blk}_{t4}_{half}", tag="rl")
                            nc.scalar.activation(rl[:], kb[:], AF.Relu)
                            rn = tmp_p.tile([128, 512], F32,
                                            name=f"rn{blk}_{t4}_{half}", tag="rn")
                            nc.scalar.activation(rn[:], kb[:], AF.Relu,
                                                 scale=-1.0)
                            nc.scalar.activation(rn[:], rn[:], AF.Exp,
                                                 scale=-1.0)
                            nc.gpsimd.tensor_tensor(
                                out=kp_blk[:, t4, gsl], in0=rn[:],
                                in1=rl[:], op=ALU.add)
                        else:
                            nc.vector.scalar_tensor_tensor(
                                out=v_blk[:, t4, gsl], in0=ps[:],
                                scalar=1.0 / WS, in1=bvb[:, gsl],
                                op0=ALU.mult, op1=ALU.add)

            # kv phase: accumulate all 4 blocks (8 DoubleRow k-tiles) per
            # (dc, ec) in a single PSUM group, then quantize psum -> fp8 on
            # the Act engine. No SBUF accumulation traffic at all.
            for dc in range(DCH):
                dsl = ts(dc, 128)
                for ec, eo, ew in ((0, 0, 512), (1, 512, 512),
                                   (2, 1024, 16)):
                    esl = slice(eo, eo + ew)
                    pkv = ps_kv.tile([128, 512], F32,
                                     name=f"pkv{dc}_{ec}", tag="ps_kv")
                    for blk in range(NBLK):
                        for t4p in range(2):
                            nc.tensor.matmul(
                                pkv[:, 0:ew],
                                kps[blk][:, 2 * t4p:2 * t4p + 2, dsl],
                                vls[blk][:, 2 * t4p:2 * t4p + 2, esl],
                                start=(blk == 0 and t4p == 0),
                                stop=(blk == NBLK - 1 and t4p == 1),
                                perf_mode=DR)
                    nc.scalar.activation(kvks8[:, dc, esl], pkv[:, 0:ew],
                                         AF.Copy, scale=S_KV)

        # ---- pair AllReduce of (kv | ksum) in fp8 ------------------------
        # Each core's half is quantized to e4m3 (scaled) and the collective
        # sums the halves in fp8: the reduced buffer IS kv8/ksum8. The row
        # is padded to 1040 so the DoubleRow den matmul sees 16B strides.
        # pair AllReduce of the packed fp8 kv|ksum buffer (via DRAM; SBUF
        # collectives are broken in this stack)
        kv_ci = dram.tile([128, DCH, KVW], F8, name="kv_ci")
        kv_co = dram.tile([128, DCH, KVW], F8, name="kv_co")
        nc.sync.dma_start(out=kv_ci[:], in_=kvks8[:])
        nc.gpsimd.collective_compute(
            "AllReduce", ALU.add,
            ins=[kv_ci[:]], outs=[kv_co[:]], replica_groups=GROUPS)
        # result DMAs; ksum column first so den unblocks ASAP
        nc.gpsimd.dma_start(out=kvks8_o[:, :, D:D + 1],
                            in_=kv_co[:][:, :, D:D + 1])
        nc.gpsimd.dma_start(out=kvks8_o[:, :, 0:512],
                            in_=kv_co[:][:, :, 0:512])
        nc.gpsimd.dma_start(out=kvks8_o[:, :, 512:1024],
                            in_=kv_co[:][:, :, 512:1024])

        # ---------------- P2: q/num/den/attn/LN1/transpose per tgroup -----
        with ExitStack() as p2:
            c2_p = p2.enter_context(tc.tile_pool(name="c2", bufs=1))
            xg_p = p2.enter_context(tc.tile_pool(name="xg", bufs=2))
            qp_p = p2.enter_context(tc.tile_pool(name="qp", bufs=6))
            xtok_p = p2.enter_context(tc.tile_pool(name="xtok", bufs=4))
            h1_p = p2.enter_context(tc.tile_pool(name="h1", bufs=8))
            x1_p = p2.enter_context(tc.tile_pool(name="x1", bufs=4))
            x1f_p = p2.enter_context(tc.tile_pool(name="x1f", bufs=2))
            x1T_p = p2.enter_context(tc.tile_pool(name="x1T", bufs=2))
            tmp2_p = p2.enter_context(tc.tile_pool(name="p2tmp", bufs=3))
            st_p = p2.enter_context(tc.tile_pool(name="p2stat", bufs=4))
            den_p = p2.enter_context(tc.tile_pool(name="denp", bufs=2))
            ps_proj2 = p2.enter_context(
                tc.tile_pool(name="ps_proj2", bufs=3, space="PSUM"))
            ps_den = p2.enter_context(
                tc.tile_pool(name="ps_den", bufs=1, space="PSUM"))
            ps_num = p2.enter_context(
                tc.tile_pool(name="ps_num", bufs=2, space="PSUM"))
            ps_tr = p2.enter_context(
                tc.tile_pool(name="ps_tr", bufs=2, space="PSUM"))

            # q weights split into 4 column chunks (prefetch-friendly);
            # allocated last so the pool can be popped once projections done
            wq_sc = ExitStack()
            wq_p = wq_sc.enter_context(tc.tile_pool(name="wqp", bufs=4))
            wq_c = []
            for j in range(4):
                t = wq_p.tile([128, DCH, 256], F8, name=f"wq{j}", tag="wqc")
                nc.scalar.dma_start(out=t[:], in_=wq_v[:, :, ts(j, 256)])
                wq_c.append(t)
            g1b = c2_p.tile([128, D], BF16, name="g1b")
            nc.sync.dma_start(out=g1b[:], in_=_bcast_row(nc, g1_row))
            b1b = c2_p.tile([128, D], BF16, name="b1b")
            nc.sync.dma_start(out=b1b[:], in_=_bcast_row(nc, b1_row))

            def emit_qproj(tg):
                o = tg * 512
                xg = xg_p.tile([128, DCH, 512], F8, name=f"xg{tg}", tag="xg")
                nc.sync.dma_start(out=xg[:], in_=x_ownT_v[:, :, o:o + 512])
                qp_g = qp_p.tile([128, DCH, 512], F8, name=f"qpg{tg}", tag="qp")
                for qc in range(DCH):
                    ps = ps_proj2.tile([128, 512], F32, name=f"pq{tg}_{qc}",
                                       tag="ps_proj2")
                    for dc2 in range(DCH // 2):
                        nc.tensor.matmul(
                            ps[:],
                            wq_c[qc // 2][:, 2 * dc2:2 * dc2 + 2,
                                          ts(qc % 2, 128)],
                            xg[:, 2 * dc2:2 * dc2 + 2, :],
                            start=(dc2 == 0), stop=(dc2 == DCH // 2 - 1),
                            perf_mode=DR)
                    bql = bq_s[:, qc:qc + 1]
                    bqnl = bqn_s[:, qc:qc + 1]
                    # q = ps/WS + bq (bias applied inside Act, q is
                    # feature-major);  qp = relu(q) + exp(-relu(-q))
                    rl = tmp2_p.tile([128, 512], F32, name=f"qr{tg}_{qc}",
                                     tag="qr")
                    nc.scalar.activation(rl[:], ps[:], AF.Relu,
                                         bias=bql, scale=1.0 / WS)
                    rn = tmp2_p.tile([128, 512], F32, name=f"qn{tg}_{qc}",
                                     tag="qn")
                    nc.scalar.activation(rn[:], ps[:], AF.Relu,
                                         bias=bqnl, scale=-1.0 / WS)
                    nc.scalar.activation(rn[:], rn[:], AF.Exp, scale=-1.0)
                    # early tgroups on DVE (idle during the collective);
                    # late ones on gpsimd (its queue drains post-collective)
                    eng = nc.vector if tg < 4 else nc.gpsimd
                    eng.tensor_tensor(
                        out=qp_g[:, qc, :], in0=rn[:], in1=rl[:],
                        op=ALU.add)
                return qp_g

            # emit ALL q projections up front: they have no dependency on
            # the kv AllReduce, so their matmuls + activations fill the
            # collective's latency window.
            qp_queue = [emit_qproj(t) for t in range(NTG)]
            wq_sc.close()  # free q-weight SBUF once all projections queued

            for pr in range(0, NTG, 2):
                tgs = [t for t in (pr, pr + 1) if t < NTG]
                den_cs = {}
                for tg in tgs:
                    qp_g = qp_queue[tg]
                    # den*S for whole tgroup: [1, 512] = ksum8^T @ qp8
                    pdn = ps_den.tile([1, 512], F32, name=f"pdn{tg}",
                                      tag="ps_den")
                    for dc2 in range(DCH // 2):
                        nc.tensor.matmul(
                            pdn[:], kvks8_o[:, 2 * dc2:2 * dc2 + 2, D:D + 1],
                            qp_g[:, 2 * dc2:2 * dc2 + 2, :],
                            start=(dc2 == 0), stop=(dc2 == DCH // 2 - 1),
                            perf_mode=DR)
                    den_sb = den_p.tile([1, 512], F32, name=f"dnr{tg}",
                                        tag="dnr")
                    nc.vector.tensor_scalar_add(
                        out=den_sb[:], in0=pdn[:], scalar1=ATTN_EPS * S_KV)
                    nc.vector.reciprocal(out=den_sb[:], in_=den_sb[:])
                    # den_c = 1/(den*S+eps*S) = (1/S_KV)/(den_true+eps)
                    den_c = den_p.tile([128, 4, 1], F32, name=f"dnc{tg}",
                                       tag="dnc")
                    for t4 in range(4):
                        ptd = ps_tr.tile([128, 1], F32, name=f"ptd{tg}_{t4}",
                                         tag="ps_tr")
                        nc.tensor.transpose(ptd[:], den_sb[:, ts(t4, 128)],
                                            ident2[0:1, 0:1])
                        nc.scalar.copy(out=den_c[:, t4, :], in_=ptd[:])
                    den_cs[tg] = den_c

                # num in e-chunk phases: ec0 for both tgroups hides the
                # second AllReduce chunk; ec1 follows.
                h1s = {}
                for ec in range(2):
                    esl = ts(ec, 512)
                    for tg in tgs:
                        qp_g = qp_queue[tg]
                        o = tg * 512
                        for t4 in range(4):
                            tok = o + t4 * 128
                            if ec == 0:
                                h1s[(tg, t4)] = h1_p.tile(
                                    [128, D], BF16, name=f"h1_{tg}_{t4}",
                                    tag="h1")
                            xth = xtok_p.tile([128, 512], F32,
                                              name=f"xt{tg}_{t4}_{ec}",
                                              tag="xtok")
                            nc.sync.dma_start(
                                out=xth[:],
                                in_=x_own[tok:tok + 128, esl])
                            pn = ps_num.tile([128, 512], F32,
                                             name=f"pn{tg}_{t4}_{ec}",
                                             tag="ps_num")
                            for dc2 in range(DCH // 2):
                                nc.tensor.matmul(
                                    pn[:],
                                    qp_g[:, 2 * dc2:2 * dc2 + 2, ts(t4, 128)],
                                    kvks8_o[:, 2 * dc2:2 * dc2 + 2, esl],
                                    start=(dc2 == 0),
                                    stop=(dc2 == DCH // 2 - 1),
                                    perf_mode=DR)
                            nc.vector.scalar_tensor_tensor(
                                out=h1s[(tg, t4)][:, esl], in0=pn[:],
                                scalar=den_cs[tg][:, t4, 0:1],
                                in1=xth[:], op0=ALU.mult, op1=ALU.add)

                # LN1 + transpose
                for tg in tgs:
                    o = tg * 512
                    x1ns = []
                    for t4 in range(4):
                        tok = o + t4 * 128
                        h1 = h1s[(tg, t4)]
                        stats = st_p.tile([128, 2, 6], F32,
                                          name=f"s1_{tg}_{t4}", tag="st1")
                        nc.vector.bn_stats(out=stats[:, 0, :], in_=h1[:, 0:512])
                        nc.vector.bn_stats(out=stats[:, 1, :],
                                           in_=h1[:, 512:1024])
                        mv = st_p.tile([128, 2], F32, name=f"mv1_{tg}_{t4}",
                                       tag="mv1")
                        nc.vector.bn_aggr(out=mv[:], in_=stats[:])
                        rstd = st_p.tile([128, 1], F32, name=f"rs1_{tg}_{t4}",
                                         tag="rstd1")
                        nc.scalar.activation(rstd[:], mv[:, 1:2], AF.Sqrt,
                                             bias=epsb[:])
                        nc.vector.reciprocal(out=rstd[:], in_=rstd[:])
                        x1f = x1f_p.tile([128, D], BF16, name=f"x1f_{tg}_{t4}",
                                         tag="x1f")
                        nc.vector.tensor_scalar(
                            out=x1f[:], in0=h1[:], scalar1=mv[:, 0:1],
                            scalar2=rstd[:], op0=ALU.subtract, op1=ALU.mult)
                        nc.vector.tensor_tensor(
                            out=x1f[:], in0=x1f[:], in1=g1b[:], op=ALU.mult)
                        x1n = x1_p.tile([128, D], BF16, name=f"x1_{tg}_{t4}",
                                        tag="x1")
                        nc.vector.tensor_tensor(
                            out=x1n[:], in0=x1f[:], in1=b1b[:], op=ALU.add)
                        nc.sync.dma_start(out=x1_dram[tok:tok + 128, :],
                                          in_=x1n[:])
                        x1ns.append(x1n)

                    for t4 in range(4):
                        tok = o + t4 * 128
                        x1n = x1ns[t4]
                        x1T_t = x1T_p.tile([128, DCH, 128], F8,
                                           name=f"x1T{tg}_{t4}", tag="x1T")
                        for dc in range(DCH):
                            pt = ps_tr.tile([128, 128], BF16,
                                            name=f"pt{tg}_{t4}_{dc}",
                                            tag="ps_tr")
                            nc.tensor.transpose(pt[:], x1n[:, ts(dc, 128)],
                                                ident[:])
                            nc.scalar.copy(out=x1T_t[:, dc, :], in_=pt[:])
                        nc.sync.dma_start(
                            out=x1T_dram_v[:, :, tok:tok + 128], in_=x1T_t[:])

        p12.close()  # release kv/ksum accumulators before P3

        # ---------------- P3: FFN + LN2, gate/up and down pipelined -------
        # Per 512-token group: gate/up for all 22 h-chunks, then down +
        # LN2. Software-pipelined gu(0) gu(1) dn(0) gu(2) dn(1) gu(3)
        # dn(2) dn(3) so the PE never waits on the act8 chain.
        NTGH = T_OWN // 512
        with ExitStack() as p3:
            c3_p = p3.enter_context(tc.tile_pool(name="c3", bufs=1))
            wd_r = c3_p.tile([128, HCH, D], WD_DT, name="wd_r")
            for hc in range(HCH):
                nc.scalar.dma_start(out=wd_r[:, hc, :], in_=wd_v[:, hc, :])
            bdb = c3_p.tile([128, D], F32, name="bdb")
            nc.sync.dma_start(out=bdb[:], in_=_bcast_row(nc, bd_row))
            g2b = c3_p.tile([128, D], BF16, name="g2b")
            nc.sync.dma_start(out=g2b[:], in_=_bcast_row(nc, g2_row))
            b2b = c3_p.tile([128, D], BF16, name="b2b")
            nc.sync.dma_start(out=b2b[:], in_=_bcast_row(nc, b2_row))
            ffn_p = p3.enter_context(tc.tile_pool(name="ffn", bufs=3))
            x1T_q = p3.enter_context(tc.tile_pool(name="x1Tq", bufs=2))
            sg_p = p3.enter_context(tc.tile_pool(name="sg", bufs=2))
            x1r_p = p3.enter_context(tc.tile_pool(name="x1r", bufs=8))
            st3_p = p3.enter_context(tc.tile_pool(name="p3stat", bufs=4))
            out_p = p3.enter_context(tc.tile_pool(name="outp", bufs=2))
            ps_g = p3.enter_context(
                tc.tile_pool(name="ps_g", bufs=2, space="PSUM"))
            ps_u = p3.enter_context(
                tc.tile_pool(name="ps_u", bufs=2, space="PSUM"))
            ps_dn = p3.enter_context(
                tc.tile_pool(name="ps_dn", bufs=4, space="PSUM"))

            ffn_ts = {}

            def emit_gu(tgh):
                o = tgh * 512
                x1T_t = x1T_q.tile([128, DCH, 512], F8, name=f"x1Tq{tgh}",
                                   tag="x1Tq")
                nc.sync.dma_start(out=x1T_t[:],
                                  in_=x1T_dram_v[:, :, o:o + 512])
                ffn_t = ffn_p.tile([128, HCH, 512], F8 if FP8_DOWN else BF16,
                                   name=f"ffn{tgh}", tag="ffn")
                ffn_ts[tgh] = ffn_t
                for hc in range(HCH):
                    psg = ps_g.tile([128, 512], F32, name=f"pg{tgh}_{hc}",
                                    tag="ps_g")
                    for dc2 in range(DCH // 2):
                        nc.tensor.matmul(
                            psg[:],
                            wg_r[:, 2 * dc2:2 * dc2 + 2, ts(hc, 128)],
                            x1T_t[:, 2 * dc2:2 * dc2 + 2, :],
                            start=(dc2 == 0), stop=(dc2 == DCH // 2 - 1),
                            perf_mode=DR)
                    psu = ps_u.tile([128, 512], F32, name=f"pu{tgh}_{hc}",
                                    tag="ps_u")
                    for dc2 in range(DCH // 2):
                        nc.tensor.matmul(
                            psu[:],
                            wu_r[:, 2 * dc2:2 * dc2 + 2, ts(hc, 128)],
                            x1T_t[:, 2 * dc2:2 * dc2 + 2, :],
                            start=(dc2 == 0), stop=(dc2 == DCH // 2 - 1),
                            perf_mode=DR)
                    # silu(gate) on Act; act8 = (psu+16*bu)*silu
                    # (stored act is 16x true act)
                    sig = sg_p.tile([128, 512], F32, name=f"sig{tgh}_{hc}",
                                    tag="sig")
                    nc.scalar.activation(
                        sig[:], psg[:], AF.Silu,
                        bias=bg_s[:, hc:hc + 1], scale=1.0 / WS)
                    nc.vector.scalar_tensor_tensor(
                        out=ffn_t[:, hc, :], in0=psu[:],
                        scalar=bu_s[:, hc:hc + 1], in1=sig[:],
                        op0=ALU.add, op1=ALU.mult)

            def emit_down(tgh):
                ffn_t = ffn_ts[tgh]
                o = tgh * 512
                x1r = []
                for t8 in range(4):
                    tok = o + t8 * 128
                    xr = x1r_p.tile([128, D], BF16, name=f"x1r{tgh}_{t8}",
                                    tag="x1r")
                    nc.gpsimd.dma_start(out=xr[:],
                                        in_=x1_dram[tok:tok + 128, :])
                    nc.gpsimd.tensor_tensor(
                        out=xr[:], in0=xr[:], in1=bdb[:], op=ALU.add)
                    x1r.append(xr)
                for dg in range(2):
                    dsl = ts(dg, 512)
                    psd = [ps_dn.tile([128, 512], F32,
                                      name=f"pd{tgh}_{dg}_{t8}", tag="ps_dn")
                           for t8 in range(4)]
                    for hc2 in range(HCH // 2):
                        for t8 in range(4):
                            if FP8_DOWN:
                                nc.tensor.matmul(
                                    psd[t8][:],
                                    ffn_t[:, 2 * hc2:2 * hc2 + 2,
                                          ts(t8, 128)],
                                    wd_r[:, 2 * hc2:2 * hc2 + 2, dsl],
                                    start=(hc2 == 0),
                                    stop=(hc2 == HCH // 2 - 1),
                                    perf_mode=DR)
                            else:
                                for hi in range(2):
                                    hc = 2 * hc2 + hi
                                    nc.tensor.matmul(
                                        psd[t8][:],
                                        ffn_t[:, hc, ts(t8, 128)],
                                        wd_r[:, hc, dsl],
                                        start=(hc == 0),
                                        stop=(hc == HCH - 1))
                    for t8 in range(4):
                        # h2 = psd*descale + (x1 + bd)  (in-place)
                        nc.vector.scalar_tensor_tensor(
                            out=x1r[t8][:, dsl], in0=psd[t8][:],
                            scalar=DOWN_DESCALE, in1=x1r[t8][:, dsl],
                            op0=ALU.mult, op1=ALU.add)

                # LN2 + store for this group
                for t8 in range(4):
                    tok = o + t8 * 128
                    h2 = x1r[t8]
                    stats = st3_p.tile([128, 2, 6], F32,
                                       name=f"s2_{tgh}_{t8}", tag="st2")
                    nc.vector.bn_stats(out=stats[:, 0, :], in_=h2[:, 0:512])
                    nc.vector.bn_stats(out=stats[:, 1, :],
                                       in_=h2[:, 512:1024])
                    mv = st3_p.tile([128, 2], F32, name=f"mv2_{tgh}_{t8}",
                                    tag="mv2")
                    nc.vector.bn_aggr(out=mv[:], in_=stats[:])
                    rstd = st3_p.tile([128, 1], F32, name=f"rs2_{tgh}_{t8}",
                                      tag="rstd2")
                    nc.scalar.activation(rstd[:], mv[:, 1:2], AF.Sqrt,
                                         bias=epsb[:])
                    nc.vector.reciprocal(out=rstd[:], in_=rstd[:])
                    o_t = out_p.tile([128, D], BF16, name=f"o{tgh}_{t8}",
                                     tag="ot")
                    nc.vector.tensor_scalar(
                        out=o_t[:], in0=h2[:], scalar1=mv[:, 0:1],
                        scalar2=rstd[:], op0=ALU.subtract, op1=ALU.mult)
                    nc.vector.tensor_tensor(
                        out=o_t[:], in0=o_t[:], in1=g2b[:], op=ALU.mult)
                    of = out_p.tile([128, D], F32, name=f"of{tgh}_{t8}",
                                    tag="of")
                    nc.gpsimd.tensor_tensor(
                        out=of[:], in0=o_t[:], in1=b2b[:], op=ALU.add)
                    nc.sync.dma_start(out=out[tok:tok + 128, :], in_=of[:])

            emit_gu(0)
            for tgh in range(1, NTGH):
                emit_gu(tgh)
                emit_down(tgh - 1)
            emit_down(NTGH - 1)

    nc.compile()
    return nc, input_names


# ---------------------------------------------------------------------------
# Host-side wrapper
# ---------------------------------------------------------------------------

B, S, D_MODEL, D_FF = 4, 4096, 1024, 4096
FFN_H = int(2 * D_FF / 3)  # 2730

_cache = {}
LAST_RESULTS = None


def _get_program(T_OWN=2048, T_FULL=4096):
    key = (T_OWN, T_FULL)
    if key not in _cache:
        _cache[key] = build_program(T_OWN, T_FULL)
    return _cache[key]


def _prep_shared(Wqkv, bqkv, Wg, bg, Wu, bu, Wd, bd, g1, b1, g2, b2):
    f = np.float32
    Wqkv = np.asarray(Wqkv, f)
    sh = {}
    sh["wq"] = np.asarray(Wqkv[:, 0:1024] * WS, E4NP)
    sh["wk"] = np.asarray(Wqkv[:, 1024:2048] * WS, E4NP)
    sh["wv"] = np.asarray(Wqkv[:, 2048:3072] * WS, E4NP)
    bqkv = np.asarray(bqkv, f)
    sh["bq_pre"] = np.ascontiguousarray(bqkv[0:1024].reshape(8, 128).T)
    sh["bk_row"] = np.ascontiguousarray(bqkv[1024:2048].reshape(1, 1024))
    sh["bv_row"] = np.ascontiguousarray(bqkv[2048:3072].reshape(1, 1024))
    wg_p = np.zeros((1024, H_PAD), f)
    wg_p[:, :FFN_H] = np.asarray(Wg, f)
    sh["wg"] = np.asarray(wg_p * WS, E4NP)
    wu_p = np.zeros((1024, H_PAD), f)
    wu_p[:, :FFN_H] = np.asarray(Wu, f)
    sh["wu"] = np.asarray(wu_p * WS_U, E4NP)
    bg_p = np.zeros((H_PAD,), f)
    bg_p[:FFN_H] = np.asarray(bg, f)
    sh["bg_pre"] = np.ascontiguousarray(bg_p.reshape(HCH, 128).T)
    bu_p = np.zeros((H_PAD,), f)
    bu_p[:FFN_H] = np.asarray(bu, f) * WS_U  # stored bias is 16*bu
    sh["bu_pre"] = np.ascontiguousarray(bu_p.reshape(HCH, 128).T)
    wd_p = np.zeros((H_PAD, 1024), f)
    wd_p[:FFN_H, :] = np.asarray(Wd, f)
    if FP8_DOWN:
        sh["wd"] = np.asarray(wd_p * WS, E4NP)
    else:
        sh["wd"] = np.asarray(wd_p, BFNP)
    sh["bd_row"] = np.asarray(bd, f).reshape(1, 1024)
    sh["g1_row"] = np.asarray(g1, BFNP).reshape(1, 1024)
    sh["b1_row"] = np.asarray(b1, BFNP).reshape(1, 1024)
    sh["g2_row"] = np.asarray(g2, BFNP).reshape(1, 1024)
    sh["b2_row"] = np.asarray(b2, BFNP).reshape(1, 1024)
    return sh


def make_in_maps(x, Wqkv, bqkv, Wg, bg, Wu, bu, Wd, bd, g1, b1, g2, b2):
    x = np.asarray(x, np.float32)
    sh = _prep_shared(Wqkv, bqkv, Wg, bg, Wu, bu, Wd, bd, g1, b1, g2, b2)
    x8 = np.asarray(x, E4NP)
    in_maps = []
    for c in range(8):
        b, h = c // 2, c % 2
        m = dict(sh)
        m["x_ownT"] = np.ascontiguousarray(x8[b, h * 2048:(h + 1) * 2048].T)
        m["x_own"] = np.ascontiguousarray(x[b, h * 2048:(h + 1) * 2048])
        in_maps.append(m)
    return in_maps


def kernel(x, Wqkv, bqkv, Wg, bg, Wu, bu, Wd, bd, g1, b1, g2, b2):
    global LAST_RESULTS
    from concourse import bass_utils

    nc, _names = _get_program()
    in_maps = make_in_maps(x, Wqkv, bqkv, Wg, bg, Wu, bu, Wd, bd,
                           g1, b1, g2, b2)
    res = bass_utils.run_bass_kernel_spmd(nc, in_maps, core_ids=list(range(8)))
    LAST_RESULTS = res
    out = np.empty((B, S, D_MODEL), np.float32)
    for c in range(8):
        b, h = c // 2, c % 2
        out[b, h * 2048:(h + 1) * 2048] = res.results[c]["out"]
    return out


# revision 67
# speedup vs baseline: 1.0689x; 1.0138x over previous
"""Trainium2 Bass kernel for nn_Block_54219667145535 (linear-attention block).

Sharding: 8 cores, 2 per batch (B=4). Each core computes k/v projection +
[D,D] kv state on its own 2048 tokens (pair-AllReduced in bf16), and
q/attention/FFN for its half of the sequence. All large matmuls run in
fp8 e4m3 with MatmulPerfMode.DoubleRow (2 k-chunks per instruction), with
per-tensor power-of-2 scales folded into the surrounding vector ops.
"""

import os
import sys
from contextlib import ExitStack

import numpy as np


def _ensure_paths():
    for p in ("/opt/trn_rl_repo", "/root/.axon_site/_ro/trn_rl_repo"):
        if os.path.isdir(p) and p not in sys.path:
            sys.path.insert(0, p)
    try:
        import concourse.bass  # noqa: F401
    except ImportError as e:  # pragma: no cover
        raise ImportError(f"concourse not importable: {e}")


_ensure_paths()

import ml_dtypes  # noqa: E402

import concourse.bass as bass  # noqa: E402
import concourse.bacc as bacc  # noqa: E402
import concourse.tile as tile  # noqa: E402
from concourse import mybir  # noqa: E402
from concourse.bass import ts  # noqa: E402
from concourse.masks import make_identity  # noqa: E402

F32 = mybir.dt.float32
F32R = mybir.dt.float32r
BF16 = mybir.dt.bfloat16
F8 = mybir.dt.float8e4
AF = mybir.ActivationFunctionType
ALU = mybir.AluOpType
DR = mybir.MatmulPerfMode.DoubleRow

E4NP = ml_dtypes.float8_e4m3
BFNP = ml_dtypes.bfloat16

D = 1024
DCH = 8  # d chunks of 128
H_PAD = 2816
HCH = 22  # h chunks of 128
LN_EPS = 1e-5
ATTN_EPS = 1e-6

WS = 128.0    # weight scale for wq/wk/wv/wg/wd
WS_U = 16.0   # weight scale for wu (act stored as 16*act, absmax ~75)
S_KV = 1.0 / 64.0  # kv|ksum half stored as val/64 (fp8 AllReduce sums halves)
KVW = 1040         # packed kv|ksum row: 1024 kv + 1 ksum + pad to 16B mult

FP8_DOWN = True  # down matmul in fp8 (else bf16)


def _bcast_row(nc, row_ap, parts=128):
    """AP that reads a [1, N] DRAM row replicated across `parts` partitions."""
    return bass.AP(
        tensor=row_ap.tensor,
        offset=row_ap.offset,
        ap=[[0, parts]] + [list(d) for d in row_ap.ap[1:]],
    )


def build_program(T_OWN=2048, n_cores=8):
    """Build the per-core Bass/Tile program. Returns (nc, input_names)."""
    assert T_OWN % 512 == 0
    NBLK = T_OWN // 512  # P1 blocks (own tokens only)
    NTG = T_OWN // 512  # P2 tgroups
    GROUPS = [[c, c + 1] for c in range(0, n_cores, 2)]
    WD_DT = F8 if FP8_DOWN else BF16
    # h2 = psd * DOWN_DESCALE + (x1 + bd)
    DOWN_DESCALE = 1.0 / (WS_U * WS) if FP8_DOWN else 1.0 / WS_U

    nc = bacc.Bacc(
        "TRN2",
        target_bir_lowering=False,
        debug=False,
        enable_asserts=False,
        num_devices=8,
        num_swdge_queues=4,
    )

    # ---- I/O ----
    x_ownT = nc.dram_tensor("x_ownT", [D, T_OWN], F8, kind="ExternalInput").ap()
    x_own = nc.dram_tensor("x_own", [T_OWN, D], F32, kind="ExternalInput").ap()
    wq = nc.dram_tensor("wq", [D, D], F8, kind="ExternalInput").ap()
    wk = nc.dram_tensor("wk", [D, D], F8, kind="ExternalInput").ap()
    wv = nc.dram_tensor("wv", [D, D], F8, kind="ExternalInput").ap()
    bq_pre = nc.dram_tensor("bq_pre", [128, DCH], F32, kind="ExternalInput").ap()
    bk_row = nc.dram_tensor("bk_row", [1, D], F32, kind="ExternalInput").ap()
    bv_row = nc.dram_tensor("bv_row", [1, D], F32, kind="ExternalInput").ap()
    wg = nc.dram_tensor("wg", [D, H_PAD], F8, kind="ExternalInput").ap()
    wu = nc.dram_tensor("wu", [D, H_PAD], F8, kind="ExternalInput").ap()
    bg_pre = nc.dram_tensor("bg_pre", [128, HCH], F32, kind="ExternalInput").ap()
    bu_pre = nc.dram_tensor("bu_pre", [128, HCH], F32, kind="ExternalInput").ap()
    wd = nc.dram_tensor("wd", [H_PAD, D], WD_DT, kind="ExternalInput").ap()
    bd_row = nc.dram_tensor("bd_row", [1, D], F32, kind="ExternalInput").ap()
    g1_row = nc.dram_tensor("g1_row", [1, D], BF16, kind="ExternalInput").ap()
    b1_row = nc.dram_tensor("b1_row", [1, D], BF16, kind="ExternalInput").ap()
    g2_row = nc.dram_tensor("g2_row", [1, D], BF16, kind="ExternalInput").ap()
    b2_row = nc.dram_tensor("b2_row", [1, D], BF16, kind="ExternalInput").ap()
    out = nc.dram_tensor("out", [T_OWN, D], F32, kind="ExternalOutput").ap()

    input_names = [
        "x_ownT", "x_own", "wq", "wk", "wv", "bq_pre", "bk_row",
        "bv_row", "wg", "wu", "bg_pre", "bu_pre", "wd", "bd_row",
        "g1_row", "b1_row", "g2_row", "b2_row",
    ]

    # d-chunked views of DRAM (partition-inner): [(c p) t -> p c t]
    x_ownT_v = x_ownT.rearrange("(c p) t -> p c t", p=128)
    wq_v = wq.rearrange("(c p) n -> p c n", p=128)
    wk_v = wk.rearrange("(c p) n -> p c n", p=128)
    wv_v = wv.rearrange("(c p) n -> p c n", p=128)
    wg_v = wg.rearrange("(c p) n -> p c n", p=128)
    wu_v = wu.rearrange("(c p) n -> p c n", p=128)
    wd_v = wd.rearrange("(c p) n -> p c n", p=128)

    with tile.TileContext(nc) as tc, ExitStack() as top:
        dram = top.enter_context(tc.tile_pool(name="dram", bufs=1, space="DRAM"))
        x1_dram = dram.tile([T_OWN, D], BF16, name="x1_dram")

        # FFN weights are SBUF-resident for the whole kernel (fp8 makes them
        # small); their DMAs issue early in P1 and drain under P1/P2.
        wres = top.enter_context(tc.tile_pool(name="wres", bufs=1))
        wg_r = wres.tile([128, DCH, H_PAD], F8, name="wg_r")
        wu_r = wres.tile([128, DCH, H_PAD], F8, name="wu_r")
        x1T_sb = wres.tile([128, DCH, T_OWN], F8, name="x1T_sb")

        consts = top.enter_context(tc.tile_pool(name="consts", bufs=1))
        ident = consts.tile([128, 128], BF16, name="ident")
        make_identity(nc, ident[:])
        ident2 = consts.tile([2, 2], F32, name="ident2")
        make_identity(nc, ident2[:])
        epsb = consts.tile([128, 1], F32, name="epsb")
        nc.vector.memset(epsb[:], LN_EPS)
        bq_s = consts.tile([128, DCH], F32, name="bq_s")
        nc.sync.dma_start(out=bq_s[:], in_=bq_pre)
        bqn_s = consts.tile([128, DCH], F32, name="bqn_s")
        nc.scalar.activation(bqn_s[:], bq_s[:], AF.Copy, scale=-1.0)
        bg_s = consts.tile([128, HCH], F32, name="bg_s")
        nc.sync.dma_start(out=bg_s[:], in_=bg_pre)
        bu_s = consts.tile([128, HCH], F32, name="bu_s")
        nc.sync.dma_start(out=bu_s[:], in_=bu_pre)

        # kv|ksum state (live P1..P2 only). ksum rides as column 1024 of
        # kv (ones-column appended to v), pad to 1040. Accumulated fully
        # in PSUM then quantized straight to fp8.
        p12 = top.enter_context(ExitStack())
        accs = p12.enter_context(tc.tile_pool(name="accs", bufs=1))
        kvks8 = accs.tile([128, DCH, KVW], F8, name="kvks8")
        kvks8_o = accs.tile([128, DCH, KVW], F8, name="kvks8_o")

        # ---------------- P1: k/v projection + kv/ksum over own tokens ----
        with ExitStack() as p1:
            c1_p = p1.enter_context(tc.tile_pool(name="c1", bufs=1))
            wkv_p = p1.enter_context(tc.tile_pool(name="wkv", bufs=1))
            xb_p = p1.enter_context(tc.tile_pool(name="xb", bufs=4))
            kpv_p = p1.enter_context(tc.tile_pool(name="kpv", bufs=NBLK))
            tmp_p = p1.enter_context(tc.tile_pool(name="p1tmp", bufs=3))
            ps_proj = p1.enter_context(
                tc.tile_pool(name="ps_proj", bufs=4, space="PSUM"))
            ps_kv = p1.enter_context(
                tc.tile_pool(name="ps_kv", bufs=3, space="PSUM"))

            # first x block before weights so PE can start ASAP
            xblk0 = xb_p.tile([128, DCH, 512], F8, name="xblk0", tag="xblk")
            for t4 in range(4):
                nc.sync.dma_start(out=xblk0[:, :, ts(t4, 128)],
                                  in_=x_ownT_v[:, :, ts(t4, 128)])
            wh = {}
            for which, half in ((0, 0), (1, 0), (0, 1), (1, 1)):
                w_v = wk_v if which == 0 else wv_v
                nm = f"w{'k' if which == 0 else 'v'}h{half}"
                t = wkv_p.tile([128, DCH, 512], F8, name=nm)
                for dc in range(DCH):
                    nc.scalar.dma_start(
                        out=t[:, dc, :],
                        in_=w_v[:, dc, ts(half, 512)])
                wh[(which, half)] = t
            bkb = c1_p.tile([128, D], F32, name="bkb")
            nc.sync.dma_start(out=bkb[:], in_=_bcast_row(nc, bk_row))
            bvb = c1_p.tile([128, D], F32, name="bvb")
            nc.sync.dma_start(out=bvb[:], in_=_bcast_row(nc, bv_row))

            # gate/up resident-weight loads: issued now so they drain in the
            # background during P1/P2 without delaying P1's first tiles
            for dc in range(DCH):
                nc.scalar.dma_start(out=wg_r[:, dc, :], in_=wg_v[:, dc, :])
                nc.scalar.dma_start(out=wu_r[:, dc, :], in_=wu_v[:, dc, :])

            kps, vls = [], []
            for blk in range(NBLK):
                if blk == 0:
                    xblk = xblk0
                else:
                    xblk = xb_p.tile([128, DCH, 512], F8, name=f"xblk{blk}",
                                     tag="xblk")
                    nc.sync.dma_start(out=xblk[:],
                                      in_=x_ownT_v[:, :, ts(blk, 512)])

                kp_blk = kpv_p.tile([128, 4, D], F8, name=f"kp{blk}", tag="kp")
                # v gets a ones column at 1024: the kv matmul then produces
                # ksum (= kp^T @ 1) as kv column 1024 for free
                v_blk = kpv_p.tile([128, 4, KVW], F8, name=f"v{blk}", tag="v")
                kps.append(kp_blk)
                vls.append(v_blk)
                nc.vector.memset(v_blk[:, :, D:D + 1], 1.0)
                nc.vector.memset(v_blk[:, :, D + 1:KVW], 0.0)

                for t4 in range(4):
                    for which, half in ((0, 0), (1, 0), (0, 1), (1, 1)):
                        w_s = wh[(which, half)]
                        gsl = ts(half, 512)
                        ps = ps_proj.tile([128, 512], F32,
                                          name=f"pp{blk}_{t4}_{which}_{half}",
                                          tag="ps_proj")
                        for dc2 in range(DCH // 2):
                            nc.tensor.matmul(
                                ps[:], xblk[:, 2 * dc2:2 * dc2 + 2,
                                            ts(t4, 128)],
                                w_s[:, 2 * dc2:2 * dc2 + 2, :],
                                start=(dc2 == 0), stop=(dc2 == DCH // 2 - 1),
                                perf_mode=DR)
                        if which == 0:
                            # k = ps/WS + bk;  kp = relu(k) + exp(min(k,0));
                            # min(k,0) = -relu(-k) keeps it all on Act engine
                            kb = tmp_p.tile([128, 512], F32,
                                            name=f"kb{blk}_{t4}_{half}", tag="kb")
                            nc.vector.scalar_tensor_tensor(
                                out=kb[:], in0=ps[:], scalar=1.0 / WS,
                                in1=bkb[:, gsl], op0=ALU.mult, op1=ALU.add)
                            rl = tmp_p.tile([128, 512], F32,
                                            name=f"rl{blk}_{t4}_{half}", tag="rl")
                            nc.scalar.activation(rl[:], kb[:], AF.Relu)
                            rn = tmp_p.tile([128, 512], F32,
                                            name=f"rn{blk}_{t4}_{half}", tag="rn")
                            nc.scalar.activation(rn[:], kb[:], AF.Relu,
                                                 scale=-1.0)
                            nc.scalar.activation(rn[:], rn[:], AF.Exp,
                                                 scale=-1.0)
                            nc.gpsimd.tensor_tensor(
                                out=kp_blk[:, t4, gsl], in0=rn[:],
                                in1=rl[:], op=ALU.add)
                        else:
                            nc.vector.scalar_tensor_tensor(
                                out=v_blk[:, t4, gsl], in0=ps[:],
                                scalar=1.0 / WS, in1=bvb[:, gsl],
                                op0=ALU.mult, op1=ALU.add)

            # kv phase: accumulate all 4 blocks (8 DoubleRow k-tiles) per
            # (dc, ec) in a single PSUM group, then quantize psum -> fp8 on
            # the Act engine. No SBUF accumulation traffic at all.
            for dc in range(DCH):
                dsl = ts(dc, 128)
                for ec, eo, ew in ((0, 0, 512), (1, 512, 512),
                                   (2, 1024, 16)):
                    esl = slice(eo, eo + ew)
                    pkv = ps_kv.tile([128, 512], F32,
                                     name=f"pkv{dc}_{ec}", tag="ps_kv")
                    for blk in range(NBLK):
                        for t4p in range(2):
                            nc.tensor.matmul(
                                pkv[:, 0:ew],
                                kps[blk][:, 2 * t4p:2 * t4p + 2, dsl],
                                vls[blk][:, 2 * t4p:2 * t4p + 2, esl],
                                start=(blk == 0 and t4p == 0),
                                stop=(blk == NBLK - 1 and t4p == 1),
                                perf_mode=DR)
                    nc.scalar.activation(kvks8[:, dc, esl], pkv[:, 0:ew],
                                         AF.Copy, scale=S_KV)

        # ---- pair AllReduce of (kv | ksum) in fp8 ------------------------
        # Each core's half is quantized to e4m3 (scaled) and the collective
        # sums the halves in fp8: the reduced buffer IS kv8/ksum8. The row
        # is padded to 1040 so the DoubleRow den matmul sees 16B strides.
        # pair AllReduce of the packed fp8 kv|ksum buffer (via DRAM; SBUF
        # collectives are broken in this stack)
        kv_ci = dram.tile([128, DCH, KVW], F8, name="kv_ci")
        kv_co = dram.tile([128, DCH, KVW], F8, name="kv_co")
        nc.sync.dma_start(out=kv_ci[:], in_=kvks8[:])
        nc.gpsimd.collective_compute(
            "AllReduce", ALU.add,
            ins=[kv_ci[:]], outs=[kv_co[:]], replica_groups=GROUPS)
        # result DMAs; ksum column first so den unblocks ASAP
        nc.gpsimd.dma_start(out=kvks8_o[:, :, D:D + 1],
                            in_=kv_co[:][:, :, D:D + 1])
        nc.gpsimd.dma_start(out=kvks8_o[:, :, 0:512],
                            in_=kv_co[:][:, :, 0:512])
        nc.gpsimd.dma_start(out=kvks8_o[:, :, 512:1024],
                            in_=kv_co[:][:, :, 512:1024])

        # ---------------- P2: q/num/den/attn/LN1/transpose per tgroup -----
        with ExitStack() as p2:
            c2_p = p2.enter_context(tc.tile_pool(name="c2", bufs=1))
            xg_p = p2.enter_context(tc.tile_pool(name="xg", bufs=2))
            qp_p = p2.enter_context(tc.tile_pool(name="qp", bufs=6))
            xtok_p = p2.enter_context(tc.tile_pool(name="xtok", bufs=4))
            h1_p = p2.enter_context(tc.tile_pool(name="h1", bufs=8))
            x1_p = p2.enter_context(tc.tile_pool(name="x1", bufs=4))
            x1f_p = p2.enter_context(tc.tile_pool(name="x1f", bufs=2))
            tmp2_p = p2.enter_context(tc.tile_pool(name="p2tmp", bufs=3))
            st_p = p2.enter_context(tc.tile_pool(name="p2stat", bufs=4))
            den_p = p2.enter_context(tc.tile_pool(name="denp", bufs=2))
            # q-proj psum pool is scoped: closed once all projections are
            # queued so den/num get more PSUM banks
            qps_sc = ExitStack()
            ps_proj2 = qps_sc.enter_context(
                tc.tile_pool(name="ps_proj2", bufs=4, space="PSUM"))

            # q weights split into 4 column chunks (prefetch-friendly);
            # allocated last so the pool can be popped once projections done
            wq_sc = ExitStack()
            wq_p = wq_sc.enter_context(tc.tile_pool(name="wqp", bufs=4))
            wq_c = []
            for j in range(4):
                t = wq_p.tile([128, DCH, 256], F8, name=f"wq{j}", tag="wqc")
                nc.scalar.dma_start(out=t[:], in_=wq_v[:, :, ts(j, 256)])
                wq_c.append(t)
            g1b = c2_p.tile([128, D], BF16, name="g1b")
            nc.sync.dma_start(out=g1b[:], in_=_bcast_row(nc, g1_row))
            b1b = c2_p.tile([128, D], BF16, name="b1b")
            nc.sync.dma_start(out=b1b[:], in_=_bcast_row(nc, b1_row))

            def emit_qproj(tg):
                o = tg * 512
                xg = xg_p.tile([128, DCH, 512], F8, name=f"xg{tg}", tag="xg")
                nc.sync.dma_start(out=xg[:], in_=x_ownT_v[:, :, o:o + 512])
                qp_g = qp_p.tile([128, DCH, 512], F8, name=f"qpg{tg}", tag="qp")
                for qc in range(DCH):
                    ps = ps_proj2.tile([128, 512], F32, name=f"pq{tg}_{qc}",
                                       tag="ps_proj2")
                    for dc2 in range(DCH // 2):
                        nc.tensor.matmul(
                            ps[:],
                            wq_c[qc // 2][:, 2 * dc2:2 * dc2 + 2,
                                          ts(qc % 2, 128)],
                            xg[:, 2 * dc2:2 * dc2 + 2, :],
                            start=(dc2 == 0), stop=(dc2 == DCH // 2 - 1),
                            perf_mode=DR)
                    bql = bq_s[:, qc:qc + 1]
                    bqnl = bqn_s[:, qc:qc + 1]
                    # q = ps/WS + bq (bias applied inside Act, q is
                    # feature-major);  qp = relu(q) + exp(-relu(-q))
                    rl = tmp2_p.tile([128, 512], F32, name=f"qr{tg}_{qc}",
                                     tag="qr")
                    nc.scalar.activation(rl[:], ps[:], AF.Relu,
                                         bias=bql, scale=1.0 / WS)
                    rn = tmp2_p.tile([128, 512], F32, name=f"qn{tg}_{qc}",
                                     tag="qn")
                    nc.scalar.activation(rn[:], ps[:], AF.Relu,
                                         bias=bqnl, scale=-1.0 / WS)
                    nc.scalar.activation(rn[:], rn[:], AF.Exp, scale=-1.0)
                    # early tgroups on DVE (idle during the collective);
                    # late ones on gpsimd (its queue drains post-collective)
                    eng = nc.vector if tg < 4 else nc.gpsimd
                    eng.tensor_tensor(
                        out=qp_g[:, qc, :], in0=rn[:], in1=rl[:],
                        op=ALU.add)
                return qp_g

            # emit ALL q projections up front: they have no dependency on
            # the kv AllReduce, so their matmuls + activations fill the
            # collective's latency window.
            qp_queue = [emit_qproj(t) for t in range(NTG)]
            wq_sc.close()  # free q-weight SBUF once all projections queued
            qps_sc.close()  # free q-proj PSUM banks for den/num
            ps_den = p2.enter_context(
                tc.tile_pool(name="ps_den", bufs=2, space="PSUM"))
            ps_num = p2.enter_context(
                tc.tile_pool(name="ps_num", bufs=4, space="PSUM"))
            ps_tr = p2.enter_context(
                tc.tile_pool(name="ps_tr", bufs=2, space="PSUM"))

            for pr in range(0, NTG, 2):
                tgs = [t for t in (pr, pr + 1) if t < NTG]
                den_cs = {}
                for tg in tgs:
                    qp_g = qp_queue[tg]
                    # den*S for whole tgroup: [1, 512] = ksum8^T @ qp8
                    pdn = ps_den.tile([1, 512], F32, name=f"pdn{tg}",
                                      tag="ps_den")
                    for dc2 in range(DCH // 2):
                        nc.tensor.matmul(
                            pdn[:], kvks8_o[:, 2 * dc2:2 * dc2 + 2, D:D + 1],
                            qp_g[:, 2 * dc2:2 * dc2 + 2, :],
                            start=(dc2 == 0), stop=(dc2 == DCH // 2 - 1),
                            perf_mode=DR)
                    den_sb = den_p.tile([1, 512], F32, name=f"dnr{tg}",
                                        tag="dnr")
                    nc.vector.tensor_scalar_add(
                        out=den_sb[:], in0=pdn[:], scalar1=ATTN_EPS * S_KV)
                    nc.vector.reciprocal(out=den_sb[:], in_=den_sb[:])
                    # den_c = 1/(den*S+eps*S) = (1/S_KV)/(den_true+eps)
                    den_c = den_p.tile([128, 4, 1], F32, name=f"dnc{tg}",
                                       tag="dnc")
                    for t4 in range(4):
                        ptd = ps_tr.tile([128, 1], F32, name=f"ptd{tg}_{t4}",
                                         tag="ps_tr")
                        nc.tensor.transpose(ptd[:], den_sb[:, ts(t4, 128)],
                                            ident2[0:1, 0:1])
                        nc.scalar.copy(out=den_c[:, t4, :], in_=ptd[:])
                    den_cs[tg] = den_c

                # num in e-chunk phases: ec0 for both tgroups hides the
                # second AllReduce chunk; ec1 follows.
                h1s = {}
                for ec in range(2):
                    esl = ts(ec, 512)
                    for tg in tgs:
                        qp_g = qp_queue[tg]
                        o = tg * 512
                        for t4 in range(4):
                            tok = o + t4 * 128
                            if ec == 0:
                                h1s[(tg, t4)] = h1_p.tile(
                                    [128, D], BF16, name=f"h1_{tg}_{t4}",
                                    tag="h1")
                            xth = xtok_p.tile([128, 512], F32,
                                              name=f"xt{tg}_{t4}_{ec}",
                                              tag="xtok")
                            nc.sync.dma_start(
                                out=xth[:],
                                in_=x_own[tok:tok + 128, esl])
                            pn = ps_num.tile([128, 512], F32,
                                             name=f"pn{tg}_{t4}_{ec}",
                                             tag="ps_num")
                            for dc2 in range(DCH // 2):
                                nc.tensor.matmul(
                                    pn[:],
                                    qp_g[:, 2 * dc2:2 * dc2 + 2, ts(t4, 128)],
                                    kvks8_o[:, 2 * dc2:2 * dc2 + 2, esl],
                                    start=(dc2 == 0),
                                    stop=(dc2 == DCH // 2 - 1),
                                    perf_mode=DR)
                            nc.vector.scalar_tensor_tensor(
                                out=h1s[(tg, t4)][:, esl], in0=pn[:],
                                scalar=den_cs[tg][:, t4, 0:1],
                                in1=xth[:], op0=ALU.mult, op1=ALU.add)

                # LN1 + transpose
                for tg in tgs:
                    o = tg * 512
                    x1ns = []
                    for t4 in range(4):
                        tok = o + t4 * 128
                        h1 = h1s[(tg, t4)]
                        stats = st_p.tile([128, 2, 6], F32,
                                          name=f"s1_{tg}_{t4}", tag="st1")
                        nc.vector.bn_stats(out=stats[:, 0, :], in_=h1[:, 0:512])
                        nc.vector.bn_stats(out=stats[:, 1, :],
                                           in_=h1[:, 512:1024])
                        mv = st_p.tile([128, 2], F32, name=f"mv1_{tg}_{t4}",
                                       tag="mv1")
                        nc.vector.bn_aggr(out=mv[:], in_=stats[:])
                        rstd = st_p.tile([128, 1], F32, name=f"rs1_{tg}_{t4}",
                                         tag="rstd1")
                        nc.scalar.activation(rstd[:], mv[:, 1:2], AF.Sqrt,
                                             bias=epsb[:])
                        nc.vector.reciprocal(out=rstd[:], in_=rstd[:])
                        x1f = x1f_p.tile([128, D], BF16, name=f"x1f_{tg}_{t4}",
                                         tag="x1f")
                        nc.vector.tensor_scalar(
                            out=x1f[:], in0=h1[:], scalar1=mv[:, 0:1],
                            scalar2=rstd[:], op0=ALU.subtract, op1=ALU.mult)
                        nc.vector.tensor_tensor(
                            out=x1f[:], in0=x1f[:], in1=g1b[:], op=ALU.mult)
                        x1n = x1_p.tile([128, D], BF16, name=f"x1_{tg}_{t4}",
                                        tag="x1")
                        nc.vector.tensor_tensor(
                            out=x1n[:], in0=x1f[:], in1=b1b[:], op=ALU.add)
                        nc.sync.dma_start(out=x1_dram[tok:tok + 128, :],
                                          in_=x1n[:])
                        x1ns.append(x1n)

                    for t4 in range(4):
                        tok = o + t4 * 128
                        x1n = x1ns[t4]
                        for dc in range(DCH):
                            pt = ps_tr.tile([128, 128], BF16,
                                            name=f"pt{tg}_{t4}_{dc}",
                                            tag="ps_tr")
                            nc.tensor.transpose(pt[:], x1n[:, ts(dc, 128)],
                                                ident[:])
                            nc.scalar.copy(out=x1T_sb[:, dc, tok:tok + 128],
                                           in_=pt[:])

        p12.close()  # release kv/ksum accumulators before P3

        # ---------------- P3: FFN + LN2, gate/up and down pipelined -------
        # Per 512-token group: gate/up for all 22 h-chunks, then down +
        # LN2. Software-pipelined gu(0) gu(1) dn(0) gu(2) dn(1) gu(3)
        # dn(2) dn(3) so the PE never waits on the act8 chain.
        NTGH = T_OWN // 512
        with ExitStack() as p3:
            c3_p = p3.enter_context(tc.tile_pool(name="c3", bufs=1))
            wd_r = c3_p.tile([128, HCH, D], WD_DT, name="wd_r")
            for hc in range(HCH):
                nc.scalar.dma_start(out=wd_r[:, hc, :], in_=wd_v[:, hc, :])
            bdb = c3_p.tile([128, D], F32, name="bdb")
            nc.sync.dma_start(out=bdb[:], in_=_bcast_row(nc, bd_row))
            g2b = c3_p.tile([128, D], BF16, name="g2b")
            nc.sync.dma_start(out=g2b[:], in_=_bcast_row(nc, g2_row))
            b2b = c3_p.tile([128, D], BF16, name="b2b")
            nc.sync.dma_start(out=b2b[:], in_=_bcast_row(nc, b2_row))
            ffn_p = p3.enter_context(tc.tile_pool(name="ffn", bufs=3))
            sg_p = p3.enter_context(tc.tile_pool(name="sg", bufs=2))
            x1r_p = p3.enter_context(tc.tile_pool(name="x1r", bufs=8))
            st3_p = p3.enter_context(tc.tile_pool(name="p3stat", bufs=4))
            out_p = p3.enter_context(tc.tile_pool(name="outp", bufs=2))
            ps_g = p3.enter_context(
                tc.tile_pool(name="ps_g", bufs=2, space="PSUM"))
            ps_u = p3.enter_context(
                tc.tile_pool(name="ps_u", bufs=2, space="PSUM"))
            ps_dn = p3.enter_context(
                tc.tile_pool(name="ps_dn", bufs=4, space="PSUM"))

            ffn_ts = {}

            def emit_gu(tgh):
                o = tgh * 512
                x1T_t = x1T_sb[:, :, o:o + 512]
                ffn_t = ffn_p.tile([128, HCH, 512], F8 if FP8_DOWN else BF16,
                                   name=f"ffn{tgh}", tag="ffn")
                ffn_ts[tgh] = ffn_t
                for hc in range(HCH):
                    psg = ps_g.tile([128, 512], F32, name=f"pg{tgh}_{hc}",
                                    tag="ps_g")
                    for dc2 in range(DCH // 2):
                        nc.tensor.matmul(
                            psg[:],
                            wg_r[:, 2 * dc2:2 * dc2 + 2, ts(hc, 128)],
                            x1T_sb[:, 2 * dc2:2 * dc2 + 2, o:o + 512],
                            start=(dc2 == 0), stop=(dc2 == DCH // 2 - 1),
                            perf_mode=DR)
                    psu = ps_u.tile([128, 512], F32, name=f"pu{tgh}_{hc}",
                                    tag="ps_u")
                    for dc2 in range(DCH // 2):
                        nc.tensor.matmul(
                            psu[:],
                            wu_r[:, 2 * dc2:2 * dc2 + 2, ts(hc, 128)],
                            x1T_sb[:, 2 * dc2:2 * dc2 + 2, o:o + 512],
                            start=(dc2 == 0), stop=(dc2 == DCH // 2 - 1),
                            perf_mode=DR)
                    # silu(gate) on Act; act8 = (psu+16*bu)*silu
                    # (stored act is 16x true act)
                    sig = sg_p.tile([128, 512], F32, name=f"sig{tgh}_{hc}",
                                    tag="sig")
                    nc.scalar.activation(
                        sig[:], psg[:], AF.Silu,
                        bias=bg_s[:, hc:hc + 1], scale=1.0 / WS)
                    nc.vector.scalar_tensor_tensor(
                        out=ffn_t[:, hc, :], in0=psu[:],
                        scalar=bu_s[:, hc:hc + 1], in1=sig[:],
                        op0=ALU.add, op1=ALU.mult)

            def emit_down(tgh):
                ffn_t = ffn_ts[tgh]
                o = tgh * 512
                x1r = []
                for t8 in range(4):
                    tok = o + t8 * 128
                    xr = x1r_p.tile([128, D], BF16, name=f"x1r{tgh}_{t8}",
                                    tag="x1r")
                    nc.gpsimd.dma_start(out=xr[:],
                                        in_=x1_dram[tok:tok + 128, :])
                    nc.gpsimd.tensor_tensor(
                        out=xr[:], in0=xr[:], in1=bdb[:], op=ALU.add)
                    x1r.append(xr)
                for dg in range(2):
                    dsl = ts(dg, 512)
                    psd = [ps_dn.tile([128, 512], F32,
                                      name=f"pd{tgh}_{dg}_{t8}", tag="ps_dn")
                           for t8 in range(4)]
                    for hc2 in range(HCH // 2):
                        for t8 in range(4):
                            if FP8_DOWN:
                                nc.tensor.matmul(
                                    psd[t8][:],
                                    ffn_t[:, 2 * hc2:2 * hc2 + 2,
                                          ts(t8, 128)],
                                    wd_r[:, 2 * hc2:2 * hc2 + 2, dsl],
                                    start=(hc2 == 0),
                                    stop=(hc2 == HCH // 2 - 1),
                                    perf_mode=DR)
                            else:
                                for hi in range(2):
                                    hc = 2 * hc2 + hi
                                    nc.tensor.matmul(
                                        psd[t8][:],
                                        ffn_t[:, hc, ts(t8, 128)],
                                        wd_r[:, hc, dsl],
                                        start=(hc == 0),
                                        stop=(hc == HCH - 1))
                    for t8 in range(4):
                        # h2 = psd*descale + (x1 + bd)  (in-place)
                        nc.vector.scalar_tensor_tensor(
                            out=x1r[t8][:, dsl], in0=psd[t8][:],
                            scalar=DOWN_DESCALE, in1=x1r[t8][:, dsl],
                            op0=ALU.mult, op1=ALU.add)

                # LN2 + store for this group
                for t8 in range(4):
                    tok = o + t8 * 128
                    h2 = x1r[t8]
                    stats = st3_p.tile([128, 2, 6], F32,
                                       name=f"s2_{tgh}_{t8}", tag="st2")
                    nc.vector.bn_stats(out=stats[:, 0, :], in_=h2[:, 0:512])
                    nc.vector.bn_stats(out=stats[:, 1, :],
                                       in_=h2[:, 512:1024])
                    mv = st3_p.tile([128, 2], F32, name=f"mv2_{tgh}_{t8}",
                                    tag="mv2")
                    nc.vector.bn_aggr(out=mv[:], in_=stats[:])
                    rstd = st3_p.tile([128, 1], F32, name=f"rs2_{tgh}_{t8}",
                                      tag="rstd2")
                    nc.scalar.activation(rstd[:], mv[:, 1:2], AF.Sqrt,
                                         bias=epsb[:])
                    nc.vector.reciprocal(out=rstd[:], in_=rstd[:])
                    o_t = out_p.tile([128, D], BF16, name=f"o{tgh}_{t8}",
                                     tag="ot")
                    nc.vector.tensor_scalar(
                        out=o_t[:], in0=h2[:], scalar1=mv[:, 0:1],
                        scalar2=rstd[:], op0=ALU.subtract, op1=ALU.mult)
                    nc.vector.tensor_tensor(
                        out=o_t[:], in0=o_t[:], in1=g2b[:], op=ALU.mult)
                    of = out_p.tile([128, D], F32, name=f"of{tgh}_{t8}",
                                    tag="of")
                    nc.gpsimd.tensor_tensor(
                        out=of[:], in0=o_t[:], in1=b2b[:], op=ALU.add)
                    nc.sync.dma_start(out=out[tok:tok + 128, :], in_=of[:])

            emit_gu(0)
            for tgh in range(1, NTGH):
                emit_gu(tgh)
                emit_down(tgh - 1)
            emit_down(NTGH - 1)

    nc.compile()
    return nc, input_names


# ---------------------------------------------------------------------------
# Host-side wrapper
# ---------------------------------------------------------------------------

B, S, D_MODEL, D_FF = 4, 4096, 1024, 4096
FFN_H = int(2 * D_FF / 3)  # 2730

_cache = {}
LAST_RESULTS = None


def _get_program(T_OWN=2048, T_FULL=4096):
    key = (T_OWN, T_FULL)
    if key not in _cache:
        _cache[key] = build_program(T_OWN, T_FULL)
    return _cache[key]


def _prep_shared(Wqkv, bqkv, Wg, bg, Wu, bu, Wd, bd, g1, b1, g2, b2):
    f = np.float32
    Wqkv = np.asarray(Wqkv, f)
    sh = {}
    sh["wq"] = np.asarray(Wqkv[:, 0:1024] * WS, E4NP)
    sh["wk"] = np.asarray(Wqkv[:, 1024:2048] * WS, E4NP)
    sh["wv"] = np.asarray(Wqkv[:, 2048:3072] * WS, E4NP)
    bqkv = np.asarray(bqkv, f)
    sh["bq_pre"] = np.ascontiguousarray(bqkv[0:1024].reshape(8, 128).T)
    sh["bk_row"] = np.ascontiguousarray(bqkv[1024:2048].reshape(1, 1024))
    sh["bv_row"] = np.ascontiguousarray(bqkv[2048:3072].reshape(1, 1024))
    wg_p = np.zeros((1024, H_PAD), f)
    wg_p[:, :FFN_H] = np.asarray(Wg, f)
    sh["wg"] = np.asarray(wg_p * WS, E4NP)
    wu_p = np.zeros((1024, H_PAD), f)
    wu_p[:, :FFN_H] = np.asarray(Wu, f)
    sh["wu"] = np.asarray(wu_p * WS_U, E4NP)
    bg_p = np.zeros((H_PAD,), f)
    bg_p[:FFN_H] = np.asarray(bg, f)
    sh["bg_pre"] = np.ascontiguousarray(bg_p.reshape(HCH, 128).T)
    bu_p = np.zeros((H_PAD,), f)
    bu_p[:FFN_H] = np.asarray(bu, f) * WS_U  # stored bias is 16*bu
    sh["bu_pre"] = np.ascontiguousarray(bu_p.reshape(HCH, 128).T)
    wd_p = np.zeros((H_PAD, 1024), f)
    wd_p[:FFN_H, :] = np.asarray(Wd, f)
    if FP8_DOWN:
        sh["wd"] = np.asarray(wd_p * WS, E4NP)
    else:
        sh["wd"] = np.asarray(wd_p, BFNP)
    sh["bd_row"] = np.asarray(bd, f).reshape(1, 1024)
    sh["g1_row"] = np.asarray(g1, BFNP).reshape(1, 1024)
    sh["b1_row"] = np.asarray(b1, BFNP).reshape(1, 1024)
    sh["g2_row"] = np.asarray(g2, BFNP).reshape(1, 1024)
    sh["b2_row"] = np.asarray(b2, BFNP).reshape(1, 1024)
    return sh


def make_in_maps(x, Wqkv, bqkv, Wg, bg, Wu, bu, Wd, bd, g1, b1, g2, b2):
    x = np.asarray(x, np.float32)
    sh = _prep_shared(Wqkv, bqkv, Wg, bg, Wu, bu, Wd, bd, g1, b1, g2, b2)
    x8 = np.asarray(x, E4NP)
    in_maps = []
    for c in range(8):
        b, h = c // 2, c % 2
        m = dict(sh)
        m["x_ownT"] = np.ascontiguousarray(x8[b, h * 2048:(h + 1) * 2048].T)
        m["x_own"] = np.ascontiguousarray(x[b, h * 2048:(h + 1) * 2048])
        in_maps.append(m)
    return in_maps


def kernel(x, Wqkv, bqkv, Wg, bg, Wu, bu, Wd, bd, g1, b1, g2, b2):
    global LAST_RESULTS
    from concourse import bass_utils

    nc, _names = _get_program()
    in_maps = make_in_maps(x, Wqkv, bqkv, Wg, bg, Wu, bu, Wd, bd,
                           g1, b1, g2, b2)
    res = bass_utils.run_bass_kernel_spmd(nc, in_maps, core_ids=list(range(8)))
    LAST_RESULTS = res
    out = np.empty((B, S, D_MODEL), np.float32)
    for c in range(8):
        b, h = c // 2, c % 2
        out[b, h * 2048:(h + 1) * 2048] = res.results[c]["out"]
    return out


# revision 68
# speedup vs baseline: 1.0944x; 1.0238x over previous
"""Trainium2 Bass kernel for nn_Block_54219667145535 (linear-attention block).

Sharding: 8 cores, 2 per batch (B=4). Each core computes k/v projection +
[D,D] kv state on its own 2048 tokens (pair-AllReduced in bf16), and
q/attention/FFN for its half of the sequence. All large matmuls run in
fp8 e4m3 with MatmulPerfMode.DoubleRow (2 k-chunks per instruction), with
per-tensor power-of-2 scales folded into the surrounding vector ops.
"""

import os
import sys
from contextlib import ExitStack

import numpy as np


def _ensure_paths():
    for p in ("/opt/trn_rl_repo", "/root/.axon_site/_ro/trn_rl_repo"):
        if os.path.isdir(p) and p not in sys.path:
            sys.path.insert(0, p)
    try:
        import concourse.bass  # noqa: F401
    except ImportError as e:  # pragma: no cover
        raise ImportError(f"concourse not importable: {e}")


_ensure_paths()

import ml_dtypes  # noqa: E402

import concourse.bass as bass  # noqa: E402
import concourse.bacc as bacc  # noqa: E402
import concourse.tile as tile  # noqa: E402
from concourse import mybir  # noqa: E402
from concourse.bass import ts  # noqa: E402
from concourse.masks import make_identity  # noqa: E402

F32 = mybir.dt.float32
F32R = mybir.dt.float32r
BF16 = mybir.dt.bfloat16
F8 = mybir.dt.float8e4
AF = mybir.ActivationFunctionType
ALU = mybir.AluOpType
DR = mybir.MatmulPerfMode.DoubleRow

E4NP = ml_dtypes.float8_e4m3
BFNP = ml_dtypes.bfloat16

D = 1024
DCH = 8  # d chunks of 128
H_PAD = 2816
HCH = 22  # h chunks of 128
LN_EPS = 1e-5
ATTN_EPS = 1e-6

WS = 128.0    # weight scale for wq/wk/wv/wg/wd
WS_U = 16.0   # weight scale for wu (act stored as 16*act, absmax ~75)
S_KV = 1.0 / 64.0  # kv|ksum half stored as val/64 (fp8 AllReduce sums halves)
KVW = 1040         # packed kv|ksum row: 1024 kv + 1 ksum + pad to 16B mult

FP8_DOWN = True  # down matmul in fp8 (else bf16)


def _bcast_row(nc, row_ap, parts=128):
    """AP that reads a [1, N] DRAM row replicated across `parts` partitions."""
    return bass.AP(
        tensor=row_ap.tensor,
        offset=row_ap.offset,
        ap=[[0, parts]] + [list(d) for d in row_ap.ap[1:]],
    )


def build_program(T_OWN=2048, n_cores=8):
    """Build the per-core Bass/Tile program. Returns (nc, input_names)."""
    assert T_OWN % 512 == 0
    NBLK = T_OWN // 512  # P1 blocks (own tokens only)
    NTG = T_OWN // 512  # P2 tgroups
    GROUPS = [[c, c + 1] for c in range(0, n_cores, 2)]
    WD_DT = F8 if FP8_DOWN else BF16
    # h2 = psd * DOWN_DESCALE + (x1 + bd)
    DOWN_DESCALE = 1.0 / (WS_U * WS) if FP8_DOWN else 1.0 / WS_U

    nc = bacc.Bacc(
        "TRN2",
        target_bir_lowering=False,
        debug=False,
        enable_asserts=False,
        num_devices=8,
        num_swdge_queues=4,
    )

    # ---- I/O ----
    x_ownT = nc.dram_tensor("x_ownT", [D, T_OWN], F8, kind="ExternalInput").ap()
    x_own = nc.dram_tensor("x_own", [T_OWN, D], F32, kind="ExternalInput").ap()
    wq = nc.dram_tensor("wq", [D, D], F8, kind="ExternalInput").ap()
    wk = nc.dram_tensor("wk", [D, D], F8, kind="ExternalInput").ap()
    wv = nc.dram_tensor("wv", [D, D], F8, kind="ExternalInput").ap()
    bq_pre = nc.dram_tensor("bq_pre", [128, DCH], F32, kind="ExternalInput").ap()
    bk_row = nc.dram_tensor("bk_row", [1, D], F32, kind="ExternalInput").ap()
    bv_row = nc.dram_tensor("bv_row", [1, D], F32, kind="ExternalInput").ap()
    wg = nc.dram_tensor("wg", [D, H_PAD], F8, kind="ExternalInput").ap()
    wu = nc.dram_tensor("wu", [D, H_PAD], F8, kind="ExternalInput").ap()
    bg_pre = nc.dram_tensor("bg_pre", [128, HCH], F32, kind="ExternalInput").ap()
    bu_pre = nc.dram_tensor("bu_pre", [128, HCH], F32, kind="ExternalInput").ap()
    wd = nc.dram_tensor("wd", [H_PAD, D], WD_DT, kind="ExternalInput").ap()
    bd_row = nc.dram_tensor("bd_row", [1, D], F32, kind="ExternalInput").ap()
    g1_row = nc.dram_tensor("g1_row", [1, D], BF16, kind="ExternalInput").ap()
    b1_row = nc.dram_tensor("b1_row", [1, D], BF16, kind="ExternalInput").ap()
    g2_row = nc.dram_tensor("g2_row", [1, D], BF16, kind="ExternalInput").ap()
    b2_row = nc.dram_tensor("b2_row", [1, D], BF16, kind="ExternalInput").ap()
    out = nc.dram_tensor("out", [T_OWN, D], F32, kind="ExternalOutput").ap()

    input_names = [
        "x_ownT", "x_own", "wq", "wk", "wv", "bq_pre", "bk_row",
        "bv_row", "wg", "wu", "bg_pre", "bu_pre", "wd", "bd_row",
        "g1_row", "b1_row", "g2_row", "b2_row",
    ]

    # d-chunked views of DRAM (partition-inner): [(c p) t -> p c t]
    x_ownT_v = x_ownT.rearrange("(c p) t -> p c t", p=128)
    wq_v = wq.rearrange("(c p) n -> p c n", p=128)
    wk_v = wk.rearrange("(c p) n -> p c n", p=128)
    wv_v = wv.rearrange("(c p) n -> p c n", p=128)
    wg_v = wg.rearrange("(c p) n -> p c n", p=128)
    wu_v = wu.rearrange("(c p) n -> p c n", p=128)
    wd_v = wd.rearrange("(c p) n -> p c n", p=128)

    with tile.TileContext(nc) as tc, ExitStack() as top:
        dram = top.enter_context(tc.tile_pool(name="dram", bufs=1, space="DRAM"))
        x1_dram = dram.tile([T_OWN, D], BF16, name="x1_dram")

        # FFN weights are SBUF-resident for the whole kernel (fp8 makes them
        # small); their DMAs issue early in P1 and drain under P1/P2.
        wres = top.enter_context(tc.tile_pool(name="wres", bufs=1))
        wg_r = wres.tile([128, DCH, H_PAD], F8, name="wg_r")
        wu_r = wres.tile([128, DCH, H_PAD], F8, name="wu_r")
        x1T_sb = wres.tile([128, DCH, T_OWN], F8, name="x1T_sb")

        consts = top.enter_context(tc.tile_pool(name="consts", bufs=1))
        ident = consts.tile([128, 128], BF16, name="ident")
        make_identity(nc, ident[:])
        ident2 = consts.tile([2, 2], F32, name="ident2")
        make_identity(nc, ident2[:])
        epsb = consts.tile([128, 1], F32, name="epsb")
        nc.vector.memset(epsb[:], LN_EPS)
        bq_s = consts.tile([128, DCH], F32, name="bq_s")
        nc.sync.dma_start(out=bq_s[:], in_=bq_pre)
        bqn_s = consts.tile([128, DCH], F32, name="bqn_s")
        nc.scalar.activation(bqn_s[:], bq_s[:], AF.Copy, scale=-1.0)
        bg_s = consts.tile([128, HCH], F32, name="bg_s")
        nc.sync.dma_start(out=bg_s[:], in_=bg_pre)
        bu_s = consts.tile([128, HCH], F32, name="bu_s")
        nc.sync.dma_start(out=bu_s[:], in_=bu_pre)

        # kv|ksum state (live P1..P2 only). ksum rides as column 1024 of
        # kv (ones-column appended to v), pad to 1040. Accumulated fully
        # in PSUM then quantized straight to fp8.
        p12 = top.enter_context(ExitStack())
        accs = p12.enter_context(tc.tile_pool(name="accs", bufs=1))
        kvks8 = accs.tile([128, DCH, KVW], F8, name="kvks8")
        kvks8_o = accs.tile([128, DCH, KVW], F8, name="kvks8_o")

        # ---------------- P1: k/v projection + kv/ksum over own tokens ----
        with ExitStack() as p1:
            c1_p = p1.enter_context(tc.tile_pool(name="c1", bufs=1))
            wkv_p = p1.enter_context(tc.tile_pool(name="wkv", bufs=1))
            xb_p = p1.enter_context(tc.tile_pool(name="xb", bufs=4))
            kpv_p = p1.enter_context(tc.tile_pool(name="kpv", bufs=NBLK))
            tmp_p = p1.enter_context(tc.tile_pool(name="p1tmp", bufs=3))
            ps_proj = p1.enter_context(
                tc.tile_pool(name="ps_proj", bufs=4, space="PSUM"))
            ps_kv = p1.enter_context(
                tc.tile_pool(name="ps_kv", bufs=3, space="PSUM"))

            # first x block before weights so PE can start ASAP
            xblk0 = xb_p.tile([128, DCH, 512], F8, name="xblk0", tag="xblk")
            for t4 in range(4):
                nc.sync.dma_start(out=xblk0[:, :, ts(t4, 128)],
                                  in_=x_ownT_v[:, :, ts(t4, 128)])
            wh = {}
            for which, half in ((0, 0), (1, 0), (0, 1), (1, 1)):
                w_v = wk_v if which == 0 else wv_v
                nm = f"w{'k' if which == 0 else 'v'}h{half}"
                t = wkv_p.tile([128, DCH, 512], F8, name=nm)
                for dc in range(DCH):
                    nc.scalar.dma_start(
                        out=t[:, dc, :],
                        in_=w_v[:, dc, ts(half, 512)])
                wh[(which, half)] = t
            bkb = c1_p.tile([128, D], F32, name="bkb")
            nc.sync.dma_start(out=bkb[:], in_=_bcast_row(nc, bk_row))
            bvb = c1_p.tile([128, D], F32, name="bvb")
            nc.sync.dma_start(out=bvb[:], in_=_bcast_row(nc, bv_row))

            # gate/up resident-weight loads: issued now so they drain in the
            # background during P1/P2 without delaying P1's first tiles
            for dc in range(DCH):
                nc.scalar.dma_start(out=wg_r[:, dc, :], in_=wg_v[:, dc, :])
                nc.scalar.dma_start(out=wu_r[:, dc, :], in_=wu_v[:, dc, :])

            kps, vls = [], []
            for blk in range(NBLK):
                if blk == 0:
                    xblk = xblk0
                else:
                    xblk = xb_p.tile([128, DCH, 512], F8, name=f"xblk{blk}",
                                     tag="xblk")
                    nc.sync.dma_start(out=xblk[:],
                                      in_=x_ownT_v[:, :, ts(blk, 512)])

                kp_blk = kpv_p.tile([128, 4, D], F8, name=f"kp{blk}", tag="kp")
                # v gets a ones column at 1024: the kv matmul then produces
                # ksum (= kp^T @ 1) as kv column 1024 for free
                v_blk = kpv_p.tile([128, 4, KVW], F8, name=f"v{blk}", tag="v")
                kps.append(kp_blk)
                vls.append(v_blk)
                nc.vector.memset(v_blk[:, :, D:D + 1], 1.0)
                nc.vector.memset(v_blk[:, :, D + 1:KVW], 0.0)

                for t4 in range(4):
                    for which, half in ((0, 0), (1, 0), (0, 1), (1, 1)):
                        w_s = wh[(which, half)]
                        gsl = ts(half, 512)
                        ps = ps_proj.tile([128, 512], F32,
                                          name=f"pp{blk}_{t4}_{which}_{half}",
                                          tag="ps_proj")
                        for dc2 in range(DCH // 2):
                            nc.tensor.matmul(
                                ps[:], xblk[:, 2 * dc2:2 * dc2 + 2,
                                            ts(t4, 128)],
                                w_s[:, 2 * dc2:2 * dc2 + 2, :],
                                start=(dc2 == 0), stop=(dc2 == DCH // 2 - 1),
                                perf_mode=DR)
                        if which == 0:
                            # k = ps/WS + bk;  kp = relu(k) + exp(min(k,0));
                            # min(k,0) = -relu(-k) keeps it all on Act engine
                            kb = tmp_p.tile([128, 512], F32,
                                            name=f"kb{blk}_{t4}_{half}", tag="kb")
                            nc.vector.scalar_tensor_tensor(
                                out=kb[:], in0=ps[:], scalar=1.0 / WS,
                                in1=bkb[:, gsl], op0=ALU.mult, op1=ALU.add)
                            rl = tmp_p.tile([128, 512], F32,
                                            name=f"rl{blk}_{t4}_{half}", tag="rl")
                            nc.scalar.activation(rl[:], kb[:], AF.Relu)
                            rn = tmp_p.tile([128, 512], F32,
                                            name=f"rn{blk}_{t4}_{half}", tag="rn")
                            nc.scalar.activation(rn[:], kb[:], AF.Relu,
                                                 scale=-1.0)
                            nc.scalar.activation(rn[:], rn[:], AF.Exp,
                                                 scale=-1.0)
                            nc.gpsimd.tensor_tensor(
                                out=kp_blk[:, t4, gsl], in0=rn[:],
                                in1=rl[:], op=ALU.add)
                        else:
                            nc.vector.scalar_tensor_tensor(
                                out=v_blk[:, t4, gsl], in0=ps[:],
                                scalar=1.0 / WS, in1=bvb[:, gsl],
                                op0=ALU.mult, op1=ALU.add)

            # kv phase: accumulate all 4 blocks (8 DoubleRow k-tiles) per
            # (dc, ec) in a single PSUM group, then quantize psum -> fp8 on
            # the Act engine. No SBUF accumulation traffic at all.
            for dc in range(DCH):
                dsl = ts(dc, 128)
                for ec, eo, ew in ((0, 0, 512), (1, 512, 512),
                                   (2, 1024, 16)):
                    esl = slice(eo, eo + ew)
                    pkv = ps_kv.tile([128, 512], F32,
                                     name=f"pkv{dc}_{ec}", tag="ps_kv")
                    for blk in range(NBLK):
                        for t4p in range(2):
                            nc.tensor.matmul(
                                pkv[:, 0:ew],
                                kps[blk][:, 2 * t4p:2 * t4p + 2, dsl],
                                vls[blk][:, 2 * t4p:2 * t4p + 2, esl],
                                start=(blk == 0 and t4p == 0),
                                stop=(blk == NBLK - 1 and t4p == 1),
                                perf_mode=DR)
                    nc.scalar.activation(kvks8[:, dc, esl], pkv[:, 0:ew],
                                         AF.Copy, scale=S_KV)

        # ---- pair AllReduce of (kv | ksum) in fp8 ------------------------
        # Each core's half is quantized to e4m3 (scaled) and the collective
        # sums the halves in fp8: the reduced buffer IS kv8/ksum8. The row
        # is padded to 1040 so the DoubleRow den matmul sees 16B strides.
        # pair AllReduce of the packed fp8 kv|ksum buffer (via DRAM; SBUF
        # collectives are broken in this stack)
        kv_ci = dram.tile([128, DCH, KVW], F8, name="kv_ci")
        kv_co = dram.tile([128, DCH, KVW], F8, name="kv_co")
        nc.sync.dma_start(out=kv_ci[:], in_=kvks8[:])
        nc.gpsimd.collective_compute(
            "AllReduce", ALU.add,
            ins=[kv_ci[:]], outs=[kv_co[:]], replica_groups=GROUPS)
        # result DMAs; ksum column first so den unblocks ASAP
        nc.gpsimd.dma_start(out=kvks8_o[:, :, D:D + 1],
                            in_=kv_co[:][:, :, D:D + 1])
        nc.gpsimd.dma_start(out=kvks8_o[:, :, 0:512],
                            in_=kv_co[:][:, :, 0:512])
        nc.gpsimd.dma_start(out=kvks8_o[:, :, 512:1024],
                            in_=kv_co[:][:, :, 512:1024])

        # ---------------- P2: q/num/den/attn/LN1/transpose per tgroup -----
        with ExitStack() as p2:
            c2_p = p2.enter_context(tc.tile_pool(name="c2", bufs=1))
            xg_p = p2.enter_context(tc.tile_pool(name="xg", bufs=2))
            qp_p = p2.enter_context(tc.tile_pool(name="qp", bufs=6))
            xtok_p = p2.enter_context(tc.tile_pool(name="xtok", bufs=4))
            h1_p = p2.enter_context(tc.tile_pool(name="h1", bufs=8))
            x1_p = p2.enter_context(tc.tile_pool(name="x1", bufs=4))
            x1f_p = p2.enter_context(tc.tile_pool(name="x1f", bufs=2))
            tmp2_p = p2.enter_context(tc.tile_pool(name="p2tmp", bufs=3))
            st_p = p2.enter_context(tc.tile_pool(name="p2stat", bufs=4))
            den_p = p2.enter_context(tc.tile_pool(name="denp", bufs=2))
            # q-proj psum pool is scoped: closed once all projections are
            # queued so den/num get more PSUM banks
            qps_sc = ExitStack()
            ps_proj2 = qps_sc.enter_context(
                tc.tile_pool(name="ps_proj2", bufs=4, space="PSUM"))

            # q weights split into 4 column chunks (prefetch-friendly);
            # allocated last so the pool can be popped once projections done
            wq_sc = ExitStack()
            wq_p = wq_sc.enter_context(tc.tile_pool(name="wqp", bufs=4))
            wq_c = []
            for j in range(4):
                t = wq_p.tile([128, DCH, 256], F8, name=f"wq{j}", tag="wqc")
                nc.scalar.dma_start(out=t[:], in_=wq_v[:, :, ts(j, 256)])
                wq_c.append(t)
            g1b = c2_p.tile([128, D], BF16, name="g1b")
            nc.sync.dma_start(out=g1b[:], in_=_bcast_row(nc, g1_row))
            b1b = c2_p.tile([128, D], BF16, name="b1b")
            nc.sync.dma_start(out=b1b[:], in_=_bcast_row(nc, b1_row))

            def emit_qproj(tg):
                o = tg * 512
                xg = xg_p.tile([128, DCH, 512], F8, name=f"xg{tg}", tag="xg")
                nc.sync.dma_start(out=xg[:], in_=x_ownT_v[:, :, o:o + 512])
                qp_g = qp_p.tile([128, DCH, 512], F8, name=f"qpg{tg}", tag="qp")
                for qc in range(DCH):
                    ps = ps_proj2.tile([128, 512], F32, name=f"pq{tg}_{qc}",
                                       tag="ps_proj2")
                    for dc2 in range(DCH // 2):
                        nc.tensor.matmul(
                            ps[:],
                            wq_c[qc // 2][:, 2 * dc2:2 * dc2 + 2,
                                          ts(qc % 2, 128)],
                            xg[:, 2 * dc2:2 * dc2 + 2, :],
                            start=(dc2 == 0), stop=(dc2 == DCH // 2 - 1),
                            perf_mode=DR)
                    bql = bq_s[:, qc:qc + 1]
                    bqnl = bqn_s[:, qc:qc + 1]
                    # q = ps/WS + bq (bias applied inside Act, q is
                    # feature-major);  qp = relu(q) + exp(-relu(-q))
                    rl = tmp2_p.tile([128, 512], F32, name=f"qr{tg}_{qc}",
                                     tag="qr")
                    nc.scalar.activation(rl[:], ps[:], AF.Relu,
                                         bias=bql, scale=1.0 / WS)
                    rn = tmp2_p.tile([128, 512], F32, name=f"qn{tg}_{qc}",
                                     tag="qn")
                    nc.scalar.activation(rn[:], ps[:], AF.Relu,
                                         bias=bqnl, scale=-1.0 / WS)
                    nc.scalar.activation(rn[:], rn[:], AF.Exp, scale=-1.0)
                    # early tgroups on DVE (idle during the collective);
                    # late ones on gpsimd (its queue drains post-collective)
                    eng = nc.vector if tg < 4 else nc.gpsimd
                    eng.tensor_tensor(
                        out=qp_g[:, qc, :], in0=rn[:], in1=rl[:],
                        op=ALU.add)
                return qp_g

            # emit ALL q projections up front: they have no dependency on
            # the kv AllReduce, so their matmuls + activations fill the
            # collective's latency window.
            qp_queue = [emit_qproj(t) for t in range(NTG)]
            wq_sc.close()  # free q-weight SBUF once all projections queued
            qps_sc.close()  # free q-proj PSUM banks for den/num
            ps_den = p2.enter_context(
                tc.tile_pool(name="ps_den", bufs=2, space="PSUM"))
            ps_num = p2.enter_context(
                tc.tile_pool(name="ps_num", bufs=4, space="PSUM"))
            ps_tr = p2.enter_context(
                tc.tile_pool(name="ps_tr", bufs=2, space="PSUM"))

            h1s = {}

            def emit_attn(tg):
                qp_g = qp_queue[tg]
                # den*S for whole tgroup: [1, 512] = ksum8^T @ qp8
                pdn = ps_den.tile([1, 512], F32, name=f"pdn{tg}",
                                  tag="ps_den")
                for dc2 in range(DCH // 2):
                    nc.tensor.matmul(
                        pdn[:], kvks8_o[:, 2 * dc2:2 * dc2 + 2, D:D + 1],
                        qp_g[:, 2 * dc2:2 * dc2 + 2, :],
                        start=(dc2 == 0), stop=(dc2 == DCH // 2 - 1),
                        perf_mode=DR)
                den_sb = den_p.tile([1, 512], F32, name=f"dnr{tg}",
                                    tag="dnr")
                nc.vector.tensor_scalar_add(
                    out=den_sb[:], in0=pdn[:], scalar1=ATTN_EPS * S_KV)
                nc.vector.reciprocal(out=den_sb[:], in_=den_sb[:])
                # den_c = 1/(den*S+eps*S) = (1/S_KV)/(den_true+eps)
                den_c = den_p.tile([128, 4, 1], F32, name=f"dnc{tg}",
                                   tag="dnc")
                for t4 in range(4):
                    ptd = ps_tr.tile([128, 1], F32, name=f"ptd{tg}_{t4}",
                                     tag="ps_tr")
                    nc.tensor.transpose(ptd[:], den_sb[:, ts(t4, 128)],
                                        ident2[0:1, 0:1])
                    nc.scalar.copy(out=den_c[:, t4, :], in_=ptd[:])

                o = tg * 512
                for ec in range(2):
                    esl = ts(ec, 512)
                    for t4 in range(4):
                        tok = o + t4 * 128
                        if ec == 0:
                            h1s[(tg, t4)] = h1_p.tile(
                                [128, D], BF16, name=f"h1_{tg}_{t4}",
                                tag="h1")
                        xth = xtok_p.tile([128, 512], F32,
                                          name=f"xt{tg}_{t4}_{ec}",
                                          tag="xtok")
                        nc.sync.dma_start(
                            out=xth[:],
                            in_=x_own[tok:tok + 128, esl])
                        pn = ps_num.tile([128, 512], F32,
                                         name=f"pn{tg}_{t4}_{ec}",
                                         tag="ps_num")
                        for dc2 in range(DCH // 2):
                            nc.tensor.matmul(
                                pn[:],
                                qp_g[:, 2 * dc2:2 * dc2 + 2, ts(t4, 128)],
                                kvks8_o[:, 2 * dc2:2 * dc2 + 2, esl],
                                start=(dc2 == 0),
                                stop=(dc2 == DCH // 2 - 1),
                                perf_mode=DR)
                        nc.vector.scalar_tensor_tensor(
                            out=h1s[(tg, t4)][:, esl], in0=pn[:],
                            scalar=den_c[:, t4, 0:1],
                            in1=xth[:], op0=ALU.mult, op1=ALU.add)

            def emit_ln1(tg):
                o = tg * 512
                x1ns = []
                for t4 in range(4):
                    tok = o + t4 * 128
                    h1 = h1s[(tg, t4)]
                    stats = st_p.tile([128, 2, 6], F32,
                                      name=f"s1_{tg}_{t4}", tag="st1")
                    nc.vector.bn_stats(out=stats[:, 0, :], in_=h1[:, 0:512])
                    nc.vector.bn_stats(out=stats[:, 1, :],
                                       in_=h1[:, 512:1024])
                    mv = st_p.tile([128, 2], F32, name=f"mv1_{tg}_{t4}",
                                   tag="mv1")
                    nc.vector.bn_aggr(out=mv[:], in_=stats[:])
                    rstd = st_p.tile([128, 1], F32, name=f"rs1_{tg}_{t4}",
                                     tag="rstd1")
                    nc.scalar.activation(rstd[:], mv[:, 1:2], AF.Sqrt,
                                         bias=epsb[:])
                    nc.vector.reciprocal(out=rstd[:], in_=rstd[:])
                    x1f = x1f_p.tile([128, D], BF16, name=f"x1f_{tg}_{t4}",
                                     tag="x1f")
                    nc.vector.tensor_scalar(
                        out=x1f[:], in0=h1[:], scalar1=mv[:, 0:1],
                        scalar2=rstd[:], op0=ALU.subtract, op1=ALU.mult)
                    nc.vector.tensor_tensor(
                        out=x1f[:], in0=x1f[:], in1=g1b[:], op=ALU.mult)
                    x1n = x1_p.tile([128, D], BF16, name=f"x1_{tg}_{t4}",
                                    tag="x1")
                    nc.vector.tensor_tensor(
                        out=x1n[:], in0=x1f[:], in1=b1b[:], op=ALU.add)
                    nc.sync.dma_start(out=x1_dram[tok:tok + 128, :],
                                      in_=x1n[:])
                    x1ns.append(x1n)

                for t4 in range(4):
                    tok = o + t4 * 128
                    x1n = x1ns[t4]
                    for dc in range(DCH):
                        pt = ps_tr.tile([128, 128], BF16,
                                        name=f"pt{tg}_{t4}_{dc}",
                                        tag="ps_tr")
                        nc.tensor.transpose(pt[:], x1n[:, ts(dc, 128)],
                                            ident[:])
                        nc.scalar.copy(out=x1T_sb[:, dc, tok:tok + 128],
                                       in_=pt[:])

            # software pipeline: den/num of tg+1 issue ahead of the
            # DVE-heavy LN1/transpose of tg, so the PE never drains
            emit_attn(0)
            for tg in range(1, NTG):
                emit_attn(tg)
                emit_ln1(tg - 1)
            emit_ln1(NTG - 1)

        p12.close()  # release kv/ksum accumulators before P3

        # ---------------- P3: FFN + LN2, gate/up and down pipelined -------
        # Per 512-token group: gate/up for all 22 h-chunks, then down +
        # LN2. Software-pipelined gu(0) gu(1) dn(0) gu(2) dn(1) gu(3)
        # dn(2) dn(3) so the PE never waits on the act8 chain.
        NTGH = T_OWN // 512
        with ExitStack() as p3:
            c3_p = p3.enter_context(tc.tile_pool(name="c3", bufs=1))
            wd_r = c3_p.tile([128, HCH, D], WD_DT, name="wd_r")
            for hc in range(HCH):
                nc.scalar.dma_start(out=wd_r[:, hc, :], in_=wd_v[:, hc, :])
            bdb = c3_p.tile([128, D], F32, name="bdb")
            nc.sync.dma_start(out=bdb[:], in_=_bcast_row(nc, bd_row))
            g2b = c3_p.tile([128, D], BF16, name="g2b")
            nc.sync.dma_start(out=g2b[:], in_=_bcast_row(nc, g2_row))
            b2b = c3_p.tile([128, D], BF16, name="b2b")
            nc.sync.dma_start(out=b2b[:], in_=_bcast_row(nc, b2_row))
            ffn_p = p3.enter_context(tc.tile_pool(name="ffn", bufs=3))
            sg_p = p3.enter_context(tc.tile_pool(name="sg", bufs=2))
            x1r_p = p3.enter_context(tc.tile_pool(name="x1r", bufs=8))
            st3_p = p3.enter_context(tc.tile_pool(name="p3stat", bufs=4))
            out_p = p3.enter_context(tc.tile_pool(name="outp", bufs=2))
            ps_g = p3.enter_context(
                tc.tile_pool(name="ps_g", bufs=2, space="PSUM"))
            ps_u = p3.enter_context(
                tc.tile_pool(name="ps_u", bufs=2, space="PSUM"))
            ps_dn = p3.enter_context(
                tc.tile_pool(name="ps_dn", bufs=4, space="PSUM"))

            ffn_ts = {}

            def emit_gu(tgh):
                o = tgh * 512
                x1T_t = x1T_sb[:, :, o:o + 512]
                ffn_t = ffn_p.tile([128, HCH, 512], F8 if FP8_DOWN else BF16,
                                   name=f"ffn{tgh}", tag="ffn")
                ffn_ts[tgh] = ffn_t
                for hc in range(HCH):
                    psg = ps_g.tile([128, 512], F32, name=f"pg{tgh}_{hc}",
                                    tag="ps_g")
                    for dc2 in range(DCH // 2):
                        nc.tensor.matmul(
                            psg[:],
                            wg_r[:, 2 * dc2:2 * dc2 + 2, ts(hc, 128)],
                            x1T_sb[:, 2 * dc2:2 * dc2 + 2, o:o + 512],
                            start=(dc2 == 0), stop=(dc2 == DCH // 2 - 1),
                            perf_mode=DR)
                    psu = ps_u.tile([128, 512], F32, name=f"pu{tgh}_{hc}",
                                    tag="ps_u")
                    for dc2 in range(DCH // 2):
                        nc.tensor.matmul(
                            psu[:],
                            wu_r[:, 2 * dc2:2 * dc2 + 2, ts(hc, 128)],
                            x1T_sb[:, 2 * dc2:2 * dc2 + 2, o:o + 512],
                            start=(dc2 == 0), stop=(dc2 == DCH // 2 - 1),
                            perf_mode=DR)
                    # silu(gate) on Act; act8 = (psu+16*bu)*silu
                    # (stored act is 16x true act)
                    sig = sg_p.tile([128, 512], F32, name=f"sig{tgh}_{hc}",
                                    tag="sig")
                    nc.scalar.activation(
                        sig[:], psg[:], AF.Silu,
                        bias=bg_s[:, hc:hc + 1], scale=1.0 / WS)
                    nc.vector.scalar_tensor_tensor(
                        out=ffn_t[:, hc, :], in0=psu[:],
                        scalar=bu_s[:, hc:hc + 1], in1=sig[:],
                        op0=ALU.add, op1=ALU.mult)

            def emit_down(tgh):
                ffn_t = ffn_ts[tgh]
                o = tgh * 512
                x1r = []
                for t8 in range(4):
                    tok = o + t8 * 128
                    xr = x1r_p.tile([128, D], BF16, name=f"x1r{tgh}_{t8}",
                                    tag="x1r")
                    nc.gpsimd.dma_start(out=xr[:],
                                        in_=x1_dram[tok:tok + 128, :])
                    nc.gpsimd.tensor_tensor(
                        out=xr[:], in0=xr[:], in1=bdb[:], op=ALU.add)
                    x1r.append(xr)
                for dg in range(2):
                    dsl = ts(dg, 512)
                    psd = [ps_dn.tile([128, 512], F32,
                                      name=f"pd{tgh}_{dg}_{t8}", tag="ps_dn")
                           for t8 in range(4)]
                    for hc2 in range(HCH // 2):
                        for t8 in range(4):
                            if FP8_DOWN:
                                nc.tensor.matmul(
                                    psd[t8][:],
                                    ffn_t[:, 2 * hc2:2 * hc2 + 2,
                                          ts(t8, 128)],
                                    wd_r[:, 2 * hc2:2 * hc2 + 2, dsl],
                                    start=(hc2 == 0),
                                    stop=(hc2 == HCH // 2 - 1),
                                    perf_mode=DR)
                            else:
                                for hi in range(2):
                                    hc = 2 * hc2 + hi
                                    nc.tensor.matmul(
                                        psd[t8][:],
                                        ffn_t[:, hc, ts(t8, 128)],
                                        wd_r[:, hc, dsl],
                                        start=(hc == 0),
                                        stop=(hc == HCH - 1))
                    for t8 in range(4):
                        # h2 = psd*descale + (x1 + bd)  (in-place)
                        nc.vector.scalar_tensor_tensor(
                            out=x1r[t8][:, dsl], in0=psd[t8][:],
                            scalar=DOWN_DESCALE, in1=x1r[t8][:, dsl],
                            op0=ALU.mult, op1=ALU.add)

                # LN2 + store for this group
                for t8 in range(4):
                    tok = o + t8 * 128
                    h2 = x1r[t8]
                    stats = st3_p.tile([128, 2, 6], F32,
                                       name=f"s2_{tgh}_{t8}", tag="st2")
                    nc.vector.bn_stats(out=stats[:, 0, :], in_=h2[:, 0:512])
                    nc.vector.bn_stats(out=stats[:, 1, :],
                                       in_=h2[:, 512:1024])
                    mv = st3_p.tile([128, 2], F32, name=f"mv2_{tgh}_{t8}",
                                    tag="mv2")
                    nc.vector.bn_aggr(out=mv[:], in_=stats[:])
                    rstd = st3_p.tile([128, 1], F32, name=f"rs2_{tgh}_{t8}",
                                      tag="rstd2")
                    nc.scalar.activation(rstd[:], mv[:, 1:2], AF.Sqrt,
                                         bias=epsb[:])
                    nc.vector.reciprocal(out=rstd[:], in_=rstd[:])
                    o_t = out_p.tile([128, D], BF16, name=f"o{tgh}_{t8}",
                                     tag="ot")
                    nc.vector.tensor_scalar(
                        out=o_t[:], in0=h2[:], scalar1=mv[:, 0:1],
                        scalar2=rstd[:], op0=ALU.subtract, op1=ALU.mult)
                    nc.vector.tensor_tensor(
                        out=o_t[:], in0=o_t[:], in1=g2b[:], op=ALU.mult)
                    of = out_p.tile([128, D], F32, name=f"of{tgh}_{t8}",
                                    tag="of")
                    nc.gpsimd.tensor_tensor(
                        out=of[:], in0=o_t[:], in1=b2b[:], op=ALU.add)
                    nc.sync.dma_start(out=out[tok:tok + 128, :], in_=of[:])

            emit_gu(0)
            for tgh in range(1, NTGH):
                emit_gu(tgh)
                emit_down(tgh - 1)
            emit_down(NTGH - 1)

    nc.compile()
    return nc, input_names


# ---------------------------------------------------------------------------
# Host-side wrapper
# ---------------------------------------------------------------------------

B, S, D_MODEL, D_FF = 4, 4096, 1024, 4096
FFN_H = int(2 * D_FF / 3)  # 2730

_cache = {}
LAST_RESULTS = None


def _get_program(T_OWN=2048, T_FULL=4096):
    key = (T_OWN, T_FULL)
    if key not in _cache:
        _cache[key] = build_program(T_OWN, T_FULL)
    return _cache[key]


def _prep_shared(Wqkv, bqkv, Wg, bg, Wu, bu, Wd, bd, g1, b1, g2, b2):
    f = np.float32
    Wqkv = np.asarray(Wqkv, f)
    sh = {}
    sh["wq"] = np.asarray(Wqkv[:, 0:1024] * WS, E4NP)
    sh["wk"] = np.asarray(Wqkv[:, 1024:2048] * WS, E4NP)
    sh["wv"] = np.asarray(Wqkv[:, 2048:3072] * WS, E4NP)
    bqkv = np.asarray(bqkv, f)
    sh["bq_pre"] = np.ascontiguousarray(bqkv[0:1024].reshape(8, 128).T)
    sh["bk_row"] = np.ascontiguousarray(bqkv[1024:2048].reshape(1, 1024))
    sh["bv_row"] = np.ascontiguousarray(bqkv[2048:3072].reshape(1, 1024))
    wg_p = np.zeros((1024, H_PAD), f)
    wg_p[:, :FFN_H] = np.asarray(Wg, f)
    sh["wg"] = np.asarray(wg_p * WS, E4NP)
    wu_p = np.zeros((1024, H_PAD), f)
    wu_p[:, :FFN_H] = np.asarray(Wu, f)
    sh["wu"] = np.asarray(wu_p * WS_U, E4NP)
    bg_p = np.zeros((H_PAD,), f)
    bg_p[:FFN_H] = np.asarray(bg, f)
    sh["bg_pre"] = np.ascontiguousarray(bg_p.reshape(HCH, 128).T)
    bu_p = np.zeros((H_PAD,), f)
    bu_p[:FFN_H] = np.asarray(bu, f) * WS_U  # stored bias is 16*bu
    sh["bu_pre"] = np.ascontiguousarray(bu_p.reshape(HCH, 128).T)
    wd_p = np.zeros((H_PAD, 1024), f)
    wd_p[:FFN_H, :] = np.asarray(Wd, f)
    if FP8_DOWN:
        sh["wd"] = np.asarray(wd_p * WS, E4NP)
    else:
        sh["wd"] = np.asarray(wd_p, BFNP)
    sh["bd_row"] = np.asarray(bd, f).reshape(1, 1024)
    sh["g1_row"] = np.asarray(g1, BFNP).reshape(1, 1024)
    sh["b1_row"] = np.asarray(b1, BFNP).reshape(1, 1024)
    sh["g2_row"] = np.asarray(g2, BFNP).reshape(1, 1024)
    sh["b2_row"] = np.asarray(b2, BFNP).reshape(1, 1024)
    return sh


def make_in_maps(x, Wqkv, bqkv, Wg, bg, Wu, bu, Wd, bd, g1, b1, g2, b2):
    x = np.asarray(x, np.float32)
    sh = _prep_shared(Wqkv, bqkv, Wg, bg, Wu, bu, Wd, bd, g1, b1, g2, b2)
    x8 = np.asarray(x, E4NP)
    in_maps = []
    for c in range(8):
        b, h = c // 2, c % 2
        m = dict(sh)
        m["x_ownT"] = np.ascontiguousarray(x8[b, h * 2048:(h + 1) * 2048].T)
        m["x_own"] = np.ascontiguousarray(x[b, h * 2048:(h + 1) * 2048])
        in_maps.append(m)
    return in_maps


def kernel(x, Wqkv, bqkv, Wg, bg, Wu, bu, Wd, bd, g1, b1, g2, b2):
    global LAST_RESULTS
    from concourse import bass_utils

    nc, _names = _get_program()
    in_maps = make_in_maps(x, Wqkv, bqkv, Wg, bg, Wu, bu, Wd, bd,
                           g1, b1, g2, b2)
    res = bass_utils.run_bass_kernel_spmd(nc, in_maps, core_ids=list(range(8)))
    LAST_RESULTS = res
    out = np.empty((B, S, D_MODEL), np.float32)
    for c in range(8):
        b, h = c // 2, c % 2
        out[b, h * 2048:(h + 1) * 2048] = res.results[c]["out"]
    return out
